# revision 1
# baseline (speedup 1.0000x reference)
"""DisenGCN Trainium2 kernel (8 NeuronCores, SPMD node-parallel).

Strategy (hardcoded from the problem spec):
  - Shard the 20000 nodes across 8 cores (2500/core, padded to 2560 = 20*128).
  - Weights replicated; per layer each core computes its local normalized
    embedding shard, AllGathers the full table to DRAM, then dma_gathers its
    2560*16 neighbor rows into SBUF and runs the 5 capsule-routing iterations
    on the Vector/Scalar engines (node-major layout: nodes on partitions).
  - Features are stored in (d, k) transposed capsule order (host-side weight
    permutation) so per-capsule reductions are flat prefix-halving tree adds
    and all broadcast multiplies have contiguous innermost APs (DVE 2x mode).
  - Nodes are processed in quarters so the dma_gather descriptor generation
    (GPSIMD-bound) of quarter q+1 overlaps the routing DVE work of quarter q.
  - fp16 storage/compute; fp32 for softmax denominators / norms.
"""

import os
import sys
import numpy as np

for _p in ("/opt/trn_rl_repo", "/root/.axon_site/_ro/trn_rl_repo"):
    if os.path.isdir(_p) and _p not in sys.path:
        sys.path.insert(0, _p)

import concourse.bass as bass  # noqa: E402
import concourse.tile as tile  # noqa: E402
from concourse import bacc, mybir  # noqa: E402
from concourse.bass_utils import run_bass_kernel_spmd  # noqa: E402
from concourse.library_config import mlp as mlp_lib  # noqa: E402
from concourse.masks import make_identity  # noqa: E402

FP16 = mybir.dt.float16
FP32 = mybir.dt.float32
I16 = mybir.dt.int16

N_CORES = 8
M = 16          # neighbor fanout
KD = 16         # per-capsule dim
CAPS = [8, 7, 6, 5, 4, 3]
ROUTIT = 5
GCH = 1024      # dma_gather rows per instruction (>1024 crashes the exec unit)


def perm_dk(k):
    """new position d*k+kk  <-  old feature index kk*KD+d."""
    p = np.empty(k * KD, np.int64)
    for d in range(KD):
        for kk in range(k):
            p[d * k + kk] = kk * KD + d
    return p


class Cfg:
    def __init__(self, nshard, feat, n_cores=N_CORES, caps=CAPS, routit=ROUTIT,
                 qsplit=None):
        self.n_cores = n_cores
        self.nshard = nshard
        self.np_ = ((nshard + 127) // 128) * 128
        self.nt = self.np_ // 128
        self.E = self.np_ * M
        self.feat = feat
        self.fpad = ((feat + 127) // 128) * 128
        self.fchunks = self.fpad // 128
        self.caps = caps
        self.routit = routit
        self.kmax = max(caps)
        self.ntab = n_cores * self.np_
        if qsplit is None:
            qsplit = 4 if self.nt % 4 == 0 else 1
        self.qs = qsplit                    # node quarters
        self.nth = self.nt // qsplit        # node tiles per quarter


FULL_CFG = Cfg(2500, 500)


def build_nc(cfg: Cfg):
    nc = bacc.Bacc("TRN2", target_bir_lowering=False, debug=False,
                   num_devices=cfg.n_cores)
    NT, NP, E, QS, NTh = cfg.nt, cfg.np_, cfg.E, cfg.qs, cfg.nth
    KM = cfg.kmax

    feat_t = nc.dram_tensor("feat_t", [cfg.fpad, NP], FP16, kind="ExternalInput")
    pca_wT = nc.dram_tensor("pca_wT", [cfg.fpad, 128], FP16, kind="ExternalInput")
    pca_b = nc.dram_tensor("pca_b", [128, 1], FP32, kind="ExternalInput")
    wTs, bs = [], []
    for i in range(1, len(cfg.caps)):
        fo = cfg.caps[i] * KD
        wTs.append(nc.dram_tensor(f"w{i}T", [128, fo], FP16, kind="ExternalInput"))
        bs.append(nc.dram_tensor(f"b{i}", [128, 1], FP32, kind="ExternalInput"))
    gidx_d = nc.dram_tensor("gidx", [128, E // 16], I16, kind="ExternalInput")
    outs_d = []
    fdims = [128] + [k * KD for k in cfg.caps]
    for li, f in enumerate(fdims):
        outs_d.append(nc.dram_tensor(f"y{li}", [NP, f], FP16, kind="ExternalOutput"))

    tshard = nc.dram_tensor("tshard", [NP, 128], FP16)
    table = nc.dram_tensor("table", [cfg.ntab, 128], FP16, addr_space="Shared")

    def sb(name, shape, dt):
        return nc.alloc_sbuf_tensor(name, shape, dt).ap()

    Z = sb("Z", [128, M * NT, 128], FP16)       # [q][m][j] block order
    XC = sb("XC", [128, NT, 128], FP16)
    XN = sb("XN", [128, NT, 128], FP16)
    U = sb("U", [128, NT, 128], FP16)
    U2 = sb("U2", [128, NT, 128], FP16)
    XT = sb("XT", [128, NP], FP16)
    XLT = sb("XLT", [128, NP], FP16)
    PR = sb("PR", [128, M * NTh, 128], FP16)    # per-quarter scratch
    S = sb("S", [128, M * NTh * KM], FP16)      # scores -> exp -> p (in place)
    DEN = sb("DEN", [128, M * NTh], FP32)
    RIV = sb("RIV", [128, M * NTh], FP32)
    SQ = sb("SQ", [128, NTh, 128], FP32)        # normalize scratch
    RIN = sb("RIN", [128, NTh * KM], FP32)
    RIN2 = sb("RIN2", [128, NTh * KM], FP32)
    GIDX = sb("GIDX", [128, E // 16], I16)
    IDT = sb("IDT", [128, 128], FP16)
    FT = sb("FT", [128, cfg.fchunks, NP], FP16)
    PW = sb("PW", [128, cfg.fchunks, 128], FP16)
    PB = sb("PB", [128, 1], FP32)
    WTS = [sb(f"WTS{i}", [128, cfg.caps[i] * KD], FP16)
           for i in range(1, len(cfg.caps))]
    BS = [sb(f"BS{i}", [128, 1], FP32) for i in range(1, len(cfg.caps))]

    Zb = Z.rearrange("p (b f) -> p b f", f=128) if Z.ndim == 2 else Z
    Z5 = Z.rearrange("p (q m j) f -> p q m j f", q=QS, m=M)

    with tile.TileContext(nc) as tc:
        import contextlib
        ctx = contextlib.ExitStack()
        with ctx:
            psum = ctx.enter_context(tc.tile_pool(name="psum", bufs=2, space="PSUM"))
            nc.gpsimd.load_library(mlp_lib)
            make_identity(nc, IDT)

            nc.sync.dma_start(GIDX, gidx_d.ap())
            nc.sync.dma_start(FT, feat_t.ap().rearrange("(c p) n -> p c n", p=128))
            nc.sync.dma_start(PW, pca_wT.ap().rearrange("(c p) f -> p c f", p=128))
            nc.sync.dma_start(PB, pca_b.ap())
            for i in range(len(cfg.caps) - 1):
                nc.sync.dma_start(WTS[i], wTs[i].ap())
                nc.sync.dma_start(BS[i], bs[i].ap())

            def transpose_block(dst_ap, src_ap, fin, fout):
                pt = psum.tile([128, 128], FP16, tag="pt")
                nc.tensor.transpose(pt[:fout, :fin], src_ap, IDT[:fin, :fin])
                nc.scalar.copy(dst_ap, pt[:fout, :fin])

            # ---- PCA: XLT = relu(pca_w @ feat + b), then node-major XC ----
            nchunk = max(1, NP // 512)
            csz = NP // nchunk
            for c in range(nchunk):
                pl = psum.tile([128, csz], FP32, tag="pl")
                for q in range(cfg.fchunks):
                    nc.tensor.matmul(
                        pl[:, :], PW[:, q, :], FT[:, q, c * csz:(c + 1) * csz],
                        start=(q == 0), stop=(q == cfg.fchunks - 1))
                nc.scalar.activation(
                    XLT[:, c * csz:(c + 1) * csz], pl[:, :],
                    mybir.ActivationFunctionType.Relu, bias=PB[:, :], scale=1.0)
            for j in range(NT):
                transpose_block(XC[:, j, :], XLT[:, j * 128:(j + 1) * 128], 128, 128)
            nc.gpsimd.dma_start(
                outs_d[0].ap().rearrange("(p j) f -> p (j f)", p=128), XC)

            def normalize(src, dst, q, k):
                """per-capsule l2 normalize, quarter q, (d,k)-packed rows."""
                f = k * KD
                qs, qe = q * NTh, (q + 1) * NTh
                nc.scalar.activation(SQ[:, :, :f], src[:, qs:qe, :f],
                                     mybir.ActivationFunctionType.Square)
                cur = f
                rin = RIN[:, :NTh * k]
                rin2 = RIN2[:, :NTh * k]
                while cur > k:
                    h = cur // 2
                    if h > k:
                        nc.vector.tensor_tensor(
                            out=SQ[:, :, :h], in0=SQ[:, :, :h],
                            in1=SQ[:, :, h:cur], op=mybir.AluOpType.add)
                    else:
                        nc.vector.tensor_tensor(
                            out=rin.rearrange("p (j k) -> p j k", k=k),
                            in0=SQ[:, :, :h], in1=SQ[:, :, h:cur],
                            op=mybir.AluOpType.add)
                    cur = h
                nc.vector.tensor_scalar_max(rin, rin, 1e-24)
                nc.vector.reciprocal_approx_fast(rin2, rin)
                nc.scalar.sqrt(rin2, rin2)
                if dst is not None:
                    nc.vector.tensor_tensor(
                        out=dst[:, qs:qe, :f].rearrange(
                            "p j (d k) -> p j d k", k=k),
                        in0=src[:, qs:qe, :f].rearrange(
                            "p j (d k) -> p j d k", k=k),
                        in1=rin2.rearrange("p (j k) -> p j k", k=k).unsqueeze(2)
                            .broadcast_to([128, NTh, KD, k]),
                        op=mybir.AluOpType.mult)

            # ---- routing layers -------------------------------------------
            for li, k in enumerate(cfg.caps):
                f = k * KD
                if li > 0:
                    fin = cfg.caps[li - 1] * KD
                    for j in range(NT):
                        transpose_block(XT[:fin, j * 128:(j + 1) * 128],
                                        XC[:, j, :fin], 128, fin)
                    for c in range(nchunk):
                        pl = psum.tile([128, csz], FP32, tag="pl")
                        nc.tensor.matmul(
                            pl[:f, :], WTS[li - 1][:fin, :f],
                            XT[:fin, c * csz:(c + 1) * csz],
                            start=True, stop=True)
                        nc.scalar.activation(
                            XLT[:f, c * csz:(c + 1) * csz], pl[:f, :],
                            mybir.ActivationFunctionType.Identity,
                            bias=BS[li - 1][:f, :], scale=1.0)
                    for j in range(NT):
                        transpose_block(U2[:, j, :f],
                                        XLT[:f, j * 128:(j + 1) * 128], f, 128)
                    xin = U2
                else:
                    xin = XC
                for q in range(QS):
                    normalize(xin, XN, q, k)

                nc.sync.dma_start(
                    tshard.ap().rearrange("(p j) f -> p (j f)", p=128), XN)
                nc.gpsimd.collective_compute(
                    "AllGather", mybir.AluOpType.bypass,
                    replica_groups=[list(range(cfg.n_cores))],
                    ins=[tshard.ap()], outs=[table.ap()])
                for g in range(E // GCH):
                    nc.gpsimd.dma_gather(
                        Zb[:, g * (GCH // 128):(g + 1) * (GCH // 128), :],
                        table.ap(), GIDX[:, g * (GCH // 16):(g + 1) * (GCH // 16)],
                        GCH, GCH, 128)

                B = M * NTh
                PRm = PR.rearrange("p (m j) f -> p m j f", m=M)
                for q in range(QS):
                    qs, qe = q * NTh, (q + 1) * NTh
                    zq = Zb[:, q * B:(q + 1) * B, :f]            # [128, b, f]
                    zq_dk = zq.rearrange("p b (d k) -> p b d k", k=k)
                    sq_ = S[:, :B * k].rearrange("p (b k) -> p b k", k=k)
                    sq_flat = S[:, :B * k]
                    sq4 = S[:, :B * k].rearrange(
                        "p (m j k) -> p m j k", m=M, k=k)
                    for t in range(cfg.routit):
                        usrc = XN if t == 0 else U2
                        # scores: PR = z * u ; tree-reduce over d -> S
                        nc.vector.tensor_tensor(
                            out=PRm[:, :, :, :f], in0=zq.rearrange(
                                "p (m j) f -> p m j f", m=M),
                            in1=usrc[:, qs:qe, :f].unsqueeze(1).broadcast_to(
                                [128, M, NTh, f]),
                            op=mybir.AluOpType.mult)
                        cur = f
                        while cur > k:
                            h = cur // 2
                            dst = PR[:, :, :h] if h > k else sq_
                            nc.vector.tensor_tensor(
                                out=dst, in0=PR[:, :, :h],
                                in1=PR[:, :, h:cur], op=mybir.AluOpType.add)
                            cur = h
                        if t > 0:
                            # u was left unnormalized; scale scores by 1/||u||
                            nc.vector.tensor_tensor(
                                out=sq4, in0=sq4,
                                in1=RIN2[:, :NTh * k].rearrange(
                                    "p (j k) -> p j k", k=k).unsqueeze(1)
                                    .broadcast_to([128, M, NTh, k]),
                                op=mybir.AluOpType.mult)
                        # softmax over k (scores bounded; no max shift)
                        nc.scalar.activation(sq_flat, sq_flat,
                                             mybir.ActivationFunctionType.Exp)
                        nc.vector.tensor_reduce(
                            out=DEN, in_=sq_,
                            op=mybir.AluOpType.add, axis=mybir.AxisListType.X)
                        nc.vector.reciprocal_approx_fast(RIV, DEN)
                        nc.vector.tensor_tensor(
                            out=sq_, in0=sq_,
                            in1=RIV.unsqueeze(2).broadcast_to([128, B, k]),
                            op=mybir.AluOpType.mult)
                        # aggregate: PR = z * p ; tree over m ; + x_norm
                        nc.vector.tensor_tensor(
                            out=PR[:, :, :f].rearrange(
                                "p b (d k) -> p b d k", k=k),
                            in0=zq_dk,
                            in1=sq_.unsqueeze(2).broadcast_to([128, B, KD, k]),
                            op=mybir.AluOpType.mult)
                        cm = M
                        while cm > 1:
                            h = cm // 2
                            nc.vector.tensor_tensor(
                                out=PRm[:, :h, :, :f], in0=PRm[:, :h, :, :f],
                                in1=PRm[:, h:cm, :, :f], op=mybir.AluOpType.add)
                            cm = h
                        nc.vector.tensor_tensor(
                            out=U2[:, qs:qe, :f], in0=PRm[:, 0, :, :f],
                            in1=XN[:, qs:qe, :f], op=mybir.AluOpType.add)
                        if t < cfg.routit - 1:
                            normalize(U2, None, q, k)
                        else:
                            nc.scalar.activation(
                                XC[:, qs:qe, :f], U2[:, qs:qe, :f],
                                mybir.ActivationFunctionType.Relu)
                nc.gpsimd.dma_start(
                    outs_d[li + 1].ap().rearrange("(p j) f -> p (j f)", p=128),
                    XC[:, :, :f])

    nc.compile()
    return nc


# ----------------------------------------------------------------------------
# Host-side prep / assembly
# ----------------------------------------------------------------------------

def prepare_in_maps(cfg: Cfg, feature, neighbor_id, pca_w, pca_b, ws, bs_):
    NS, NP, NT, QS, NTh = cfg.nshard, cfg.np_, cfg.nt, cfg.qs, cfg.nth
    nb = np.asarray(neighbor_id).astype(np.int64)
    perms = [perm_dk(k) for k in cfg.caps]           # routing layers 0..5
    p0 = perms[0]

    pwt = np.zeros((cfg.fpad, 128), np.float16)
    pwt[:cfg.feat, :] = np.asarray(pca_w).T[:, p0].astype(np.float16)
    pbb = np.zeros((128, 1), np.float32)
    pbb[:, 0] = np.asarray(pca_b, np.float32)[p0]
    wts, bss = [], []
    for i, (w, b) in enumerate(zip(ws, bs_)):
        fo, fi = w.shape
        wp = np.asarray(w)[perms[i + 1]][:, perms[i]]   # out-perm, in-perm
        wt = np.zeros((128, fo), np.float16)
        wt[:fi, :] = wp.T.astype(np.float16)
        wts.append(wt)
        bb = np.zeros((128, 1), np.float32)
        bb[:fo, 0] = np.asarray(b, np.float32)[perms[i + 1]]
        bss.append(bb)

    def table_row(G):
        c, n = np.divmod(G, NS)
        return c * NP + (n % 128) * NT + n // 128

    in_maps = []
    for c in range(cfg.n_cores):
        lo = c * NS
        ft = np.zeros((cfg.fpad, NP), np.float16)
        ft[:cfg.feat, :NS] = np.asarray(feature[lo:lo + NS]).T.astype(np.float16)

        rows = np.zeros((NP, M), np.int64)
        rows[:NS] = table_row(nb[lo:lo + NS, :])
        # gather order: quarter-major, then m, then node tile (j), then p
        # node n = (q*NTh + j)*128 + p ; idx position = ((q*M + m)*NTh + j)*128 + p
        r4 = rows.reshape(QS, NTh * 128, M)              # [q, n_in_q, m]
        gidx = r4.transpose(0, 2, 1).reshape(-1).astype(np.int16)
        gidx_w = np.tile(gidx.reshape(-1, 16).T, (8, 1))

        m = {"feat_t": ft, "pca_wT": pwt, "pca_b": pbb, "gidx": gidx_w}
        for i in range(len(wts)):
            m[f"w{i + 1}T"] = wts[i]
            m[f"b{i + 1}"] = bss[i]
        in_maps.append(m)
    return in_maps


def assemble_output(cfg: Cfg, results):
    NS, NT = cfg.nshard, cfg.nt
    fdims = [128] + [k * KD for k in cfg.caps]
    perms = [perm_dk(k) for k in [8] + list(cfg.caps)]
    cols = []
    for li, f in enumerate(fdims):
        perm = perms[li]
        shards = []
        for c in range(cfg.n_cores):
            a = np.asarray(results[c][f"y{li}"]).astype(np.float32)
            a = a.reshape(128, NT, f).transpose(1, 0, 2).reshape(cfg.np_, f)
            u = np.empty_like(a)
            u[:, perm] = a                                # undo (d,k) packing
            shards.append(u[:NS])
        cols.append(np.concatenate(shards, axis=0))
    return np.concatenate(cols, axis=1)


def _ensure_ntff_hook():
    try:
        from antenv.axon_hooks import get_axon_ntff_profile_hook  # noqa: F401
        return True
    except ImportError:
        pass
    try:
        import types
        import antenv
        from trn_agent_boot.trn_boot import _ntff_profile_via_ctypes
        mod = types.ModuleType("antenv.axon_hooks")
        state = {"h": None}
        mod.set_axon_ntff_profile_hook = lambda h: state.__setitem__("h", h)
        mod.get_axon_ntff_profile_hook = lambda: state["h"]
        sys.modules["antenv.axon_hooks"] = mod
        antenv.axon_hooks = mod
        mod.set_axon_ntff_profile_hook(
            _ntff_profile_via_ctypes("/opt/axon/libaxon_pjrt.so"))
        return True
    except Exception:
        return False


_CACHE = {}


def _get_nc(cfg: Cfg):
    key = (cfg.nshard, cfg.feat, cfg.n_cores)
    if key not in _CACHE:
        _CACHE[key] = build_nc(cfg)
    return _CACHE[key]


def kernel(feature, neighbor_id, pca_w, pca_b,
           w1, b1, w2, b2, w3, b3, w4, b4, w5, b5):
    cfg = FULL_CFG
    nc = _get_nc(cfg)
    in_maps = prepare_in_maps(
        cfg, np.asarray(feature), np.asarray(neighbor_id),
        np.asarray(pca_w), np.asarray(pca_b),
        [np.asarray(w) for w in (w1, w2, w3, w4, w5)],
        [np.asarray(b) for b in (b1, b2, b3, b4, b5)])
    trace = bool(int(os.environ.get("KERNEL_TRACE", "0")))
    if trace:
        trace = _ensure_ntff_hook()
    tmpdir = os.environ.get("KERNEL_TRACE_DIR") or None
    res = run_bass_kernel_spmd(nc, in_maps, core_ids=list(range(cfg.n_cores)),
                               trace=trace, tmpdir=tmpdir)
    out = assemble_output(cfg, res.results)
    if trace:
        kernel.last_exec_time_ns = res.exec_time_ns
    return out


kernel.last_exec_time_ns = None



# revision 3
# speedup vs baseline: 1.0590x; 1.0590x over previous
"""DisenGCN Trainium2 kernel (8 NeuronCores, SPMD node-parallel).

Strategy (hardcoded from the problem spec):
  - Shard the 20000 nodes across 8 cores (2500/core, padded to 2560 = 20*128).
  - Weights replicated; per layer each core computes its local normalized
    embedding shard, AllGathers the full table to DRAM, then dma_gathers its
    neighbor rows into SBUF and runs the 5 capsule-routing iterations on the
    Vector/Scalar engines (node-major layout: nodes on partitions).
  - Features are stored in (d, k) transposed capsule order (host-side weight
    permutation) so per-capsule reductions are flat prefix-halving tree adds
    and all broadcast multiplies have contiguous innermost APs (DVE 2x mode).
  - Nodes are processed in 6 chunks of [2,4,4,4,4,2] tiles so the dma_gather
    descriptor generation (GPSIMD-bound, ~8us/1024 rows) of chunk c+1
    overlaps the routing DVE work of chunk c, and the per-layer tail
    (linear -> normalize -> tshard -> AllGather -> first gather) is short.
  - The AllGather is split in two pieces (tiles 0-17 / 18-19, j-major table
    rows) so the first piece overlaps the last chunks' routing.
  - 1/sqrt(ss) is computed as exp(-0.5*ln(ss+1e-12)) on the ACT engine: Ln,
    Exp, Square, Relu, Identity all live in one activation table set
    (natural_log_exp_and_others), so the inner loop never reloads tables.
  - fp16 storage/compute; fp32 only for softmax denominators and ln output.
"""

import os
import sys
import numpy as np

for _p in ("/opt/trn_rl_repo", "/root/.axon_site/_ro/trn_rl_repo"):
    if os.path.isdir(_p) and _p not in sys.path:
        sys.path.insert(0, _p)

import concourse.bass as bass  # noqa: E402
import concourse.tile as tile  # noqa: E402
from concourse import bacc, mybir  # noqa: E402
from concourse.bass_utils import run_bass_kernel_spmd  # noqa: E402
from concourse.library_config import mlp as mlp_lib  # noqa: E402
from concourse.masks import make_identity  # noqa: E402

FP16 = mybir.dt.float16
FP32 = mybir.dt.float32
I16 = mybir.dt.int16
AF = mybir.ActivationFunctionType

N_CORES = 8
M = 16          # neighbor fanout
KD = 16         # per-capsule dim
CAPS = [8, 7, 6, 5, 4, 3]
ROUTIT = 5
GCH = 1024      # dma_gather rows per instruction (>1024 overflows the ring)

CH_TILES = [2, 4, 4, 4, 4, 2]        # chunk sizes (tiles of 128 nodes)
AG_SPLIT = 18                        # tiles [0,18) = piece A, [18,20) = B


def perm_dk(k):
    """new position d*k+kk  <-  old feature index kk*KD+d."""
    p = np.empty(k * KD, np.int64)
    for d in range(KD):
        for kk in range(k):
            p[d * k + kk] = kk * KD + d
    return p


class Cfg:
    def __init__(self, nshard, feat, n_cores=N_CORES, caps=CAPS, routit=ROUTIT):
        self.n_cores = n_cores
        self.nshard = nshard
        self.np_ = ((nshard + 127) // 128) * 128
        self.nt = self.np_ // 128
        self.E = self.np_ * M
        self.feat = feat
        self.fpad = ((feat + 127) // 128) * 128
        self.fchunks = self.fpad // 128
        self.caps = caps
        self.routit = routit
        self.kmax = max(caps)
        self.ntab = n_cores * self.np_
        assert sum(CH_TILES) == self.nt
        self.ch_tiles = CH_TILES
        self.tbase = np.concatenate([[0], np.cumsum(CH_TILES)])[:-1]
        self.boff = np.concatenate([[0], np.cumsum([M * t for t in CH_TILES])])
        self.tmax = max(CH_TILES)
        # j-major table rows, split at AG_SPLIT tiles
        self.rows_a = AG_SPLIT * 128                 # per-core piece A rows
        self.rows_b = (self.nt - AG_SPLIT) * 128     # per-core piece B rows


FULL_CFG = Cfg(2500, 500)


def build_nc(cfg: Cfg):
    nc = bacc.Bacc("TRN2", target_bir_lowering=False, debug=False,
                   num_devices=cfg.n_cores)
    NT, NP, E = cfg.nt, cfg.np_, cfg.E
    KM = cfg.kmax
    TM = cfg.tmax
    BM = M * TM

    feat_t = nc.dram_tensor("feat_t", [cfg.fpad, NP], FP16, kind="ExternalInput")
    pca_wT = nc.dram_tensor("pca_wT", [cfg.fpad, 128], FP16, kind="ExternalInput")
    pca_b = nc.dram_tensor("pca_b", [128, 1], FP32, kind="ExternalInput")
    wTs, bs = [], []
    for i in range(1, len(cfg.caps)):
        fo = cfg.caps[i] * KD
        wTs.append(nc.dram_tensor(f"w{i}T", [128, fo], FP16, kind="ExternalInput"))
        bs.append(nc.dram_tensor(f"b{i}", [128, 1], FP32, kind="ExternalInput"))
    gidx_d = nc.dram_tensor("gidx", [128, E // 16], I16, kind="ExternalInput")
    outs_d = []
    fdims = [128] + [k * KD for k in cfg.caps]
    for li, f in enumerate(fdims):
        outs_d.append(nc.dram_tensor(f"y{li}", [NP, f], FP16, kind="ExternalOutput"))

    tshard = nc.dram_tensor("tshard", [NP, 128], FP16)
    table = nc.dram_tensor("table", [cfg.ntab, 128], FP16, addr_space="Shared")

    def sb(name, shape, dt):
        return nc.alloc_sbuf_tensor(name, shape, dt).ap()

    Z = sb("Z", [128, M * NT, 128], FP16)       # chunk-major [c][m][jj] blocks
    XC = sb("XC", [128, NT, 128], FP16)
    XN = sb("XN", [128, NT, 128], FP16)
    U2 = sb("U2", [128, NT, 128], FP16)
    XT = sb("XT", [128, NP], FP16)
    XLT = sb("XLT", [128, NP], FP16)
    PR = sb("PR", [128, BM, 128], FP16)         # per-chunk scratch
    S = sb("S", [128, BM * KM], FP16)           # scores -> exp -> p (in place)
    DEN = sb("DEN", [128, BM], FP32)
    RIV = sb("RIV", [128, BM], FP32)
    SQ = sb("SQ", [128, TM, 128], FP16)         # normalize scratch
    RIN = sb("RIN", [128, TM * KM], FP16)
    LNS = sb("LNS", [128, TM * KM], FP32)
    RIN2 = sb("RIN2", [128, TM * KM], FP16)
    EPS = sb("EPS", [128, 1], FP32)
    GIDX = sb("GIDX", [128, E // 16], I16)
    IDT = sb("IDT", [128, 128], FP16)
    FT = sb("FT", [128, cfg.fchunks, NP], FP16)
    PW = sb("PW", [128, cfg.fchunks, 128], FP16)
    PB = sb("PB", [128, 1], FP32)
    WTS = [sb(f"WTS{i}", [128, cfg.caps[i] * KD], FP16)
           for i in range(1, len(cfg.caps))]
    BS = [sb(f"BS{i}", [128, 1], FP32) for i in range(1, len(cfg.caps))]

    Zb = Z.rearrange("p (b f) -> p b f", f=128) if Z.ndim == 2 else Z

    with tile.TileContext(nc) as tc:
        import contextlib
        ctx = contextlib.ExitStack()
        with ctx:
            psum = ctx.enter_context(tc.tile_pool(name="psum", bufs=2, space="PSUM"))
            nc.gpsimd.load_library(mlp_lib)
            make_identity(nc, IDT)
            nc.vector.memset(EPS, 1e-12)

            nc.sync.dma_start(GIDX, gidx_d.ap())
            nc.sync.dma_start(FT, feat_t.ap().rearrange("(c p) n -> p c n", p=128))
            nc.sync.dma_start(PW, pca_wT.ap().rearrange("(c p) f -> p c f", p=128))
            nc.sync.dma_start(PB, pca_b.ap())
            for i in range(len(cfg.caps) - 1):
                nc.sync.dma_start(WTS[i], wTs[i].ap())
                nc.sync.dma_start(BS[i], bs[i].ap())

            def transpose_block(dst_ap, src_ap, fin, fout):
                pt = psum.tile([128, 128], FP16, tag="pt")
                nc.tensor.transpose(pt[:fout, :fin], src_ap, IDT[:fin, :fin])
                nc.scalar.copy(dst_ap, pt[:fout, :fin])

            def pca_chunk(c):
                t0, T = cfg.tbase[c], cfg.ch_tiles[c]
                cs, ce = t0 * 128, (t0 + T) * 128
                pl = psum.tile([128, TM * 128], FP32, tag="pl")
                for q in range(cfg.fchunks):
                    nc.tensor.matmul(
                        pl[:, :T * 128], PW[:, q, :], FT[:, q, cs:ce],
                        start=(q == 0), stop=(q == cfg.fchunks - 1))
                nc.scalar.activation(XLT[:, cs:ce], pl[:, :T * 128],
                                     AF.Relu, bias=PB[:, :], scale=1.0)
                for j in range(t0, t0 + T):
                    transpose_block(XC[:, j, :], XLT[:, j * 128:(j + 1) * 128],
                                    128, 128)

            def linear_chunk(li, c):
                # target layer li (1..5): XC (fin) -> U2 (fout)
                fin = cfg.caps[li - 1] * KD
                fout = cfg.caps[li] * KD
                t0, T = cfg.tbase[c], cfg.ch_tiles[c]
                cs, ce = t0 * 128, (t0 + T) * 128
                for j in range(t0, t0 + T):
                    transpose_block(XT[:fin, j * 128:(j + 1) * 128],
                                    XC[:, j, :fin], 128, fin)
                pl = psum.tile([128, TM * 128], FP32, tag="pl")
                nc.tensor.matmul(pl[:fout, :T * 128], WTS[li - 1][:fin, :fout],
                                 XT[:fin, cs:ce], start=True, stop=True)
                nc.scalar.activation(XLT[:fout, cs:ce], pl[:fout, :T * 128],
                                     AF.Identity, bias=BS[li - 1][:fout, :],
                                     scale=1.0)
                for j in range(t0, t0 + T):
                    transpose_block(U2[:, j, :fout], XLT[:fout, j * 128:(j + 1) * 128],
                                    fout, 128)

            def norm_stats(src, c, k):
                """RIN2 = 1/||src_chunk|| per capsule: exp(-0.5*ln(ss+eps))."""
                f = k * KD
                t0, T = cfg.tbase[c], cfg.ch_tiles[c]
                nc.scalar.activation(SQ[:, :T, :f], src[:, t0:t0 + T, :f],
                                     AF.Square)
                cur = f
                rin = RIN[:, :T * k]
                while cur > k:
                    h = cur // 2
                    if h > k:
                        nc.vector.tensor_tensor(
                            out=SQ[:, :T, :h], in0=SQ[:, :T, :h],
                            in1=SQ[:, :T, h:cur], op=mybir.AluOpType.add)
                    else:
                        nc.vector.tensor_tensor(
                            out=rin.rearrange("p (j k) -> p j k", k=k),
                            in0=SQ[:, :T, :h], in1=SQ[:, :T, h:cur],
                            op=mybir.AluOpType.add)
                    cur = h
                nc.scalar.activation(LNS[:, :T * k], rin, AF.Ln,
                                     bias=EPS[:, :], scale=1.0)
                nc.scalar.activation(RIN2[:, :T * k], LNS[:, :T * k],
                                     AF.Exp, scale=-0.5)

            def normalize_chunk(src, dst, c, k):
                """dst = per-capsule l2 normalize of src for chunk c."""
                f = k * KD
                t0, T = cfg.tbase[c], cfg.ch_tiles[c]
                norm_stats(src, c, k)
                nc.vector.tensor_tensor(
                    out=dst[:, t0:t0 + T, :f].rearrange(
                        "p j (d k) -> p j d k", k=k),
                    in0=src[:, t0:t0 + T, :f].rearrange(
                        "p j (d k) -> p j d k", k=k),
                    in1=RIN2[:, :T * k].rearrange("p (j k) -> p j k", k=k)
                        .unsqueeze(2).broadcast_to([128, T, KD, k]),
                    op=mybir.AluOpType.mult)

            def tshard_chunk(c):
                t0, T = cfg.tbase[c], cfg.ch_tiles[c]
                nc.sync.dma_start(
                    tshard.ap()[t0 * 128:(t0 + T) * 128, :]
                    .rearrange("(j p) f -> p j f", p=128),
                    XN[:, t0:t0 + T, :])

            def allgathers(c):
                if cfg.tbase[c] + cfg.ch_tiles[c] == AG_SPLIT:
                    nc.gpsimd.collective_compute(
                        "AllGather", mybir.AluOpType.bypass,
                        replica_groups=[list(range(cfg.n_cores))],
                        ins=[tshard.ap()[0:cfg.rows_a, :]],
                        outs=[table.ap()[0:cfg.n_cores * cfg.rows_a, :]])
                if cfg.tbase[c] + cfg.ch_tiles[c] == NT:
                    nc.gpsimd.collective_compute(
                        "AllGather", mybir.AluOpType.bypass,
                        replica_groups=[list(range(cfg.n_cores))],
                        ins=[tshard.ap()[cfg.rows_a:NP, :]],
                        outs=[table.ap()[cfg.n_cores * cfg.rows_a:cfg.ntab, :]])

            def gathers():
                for c in range(len(cfg.ch_tiles)):
                    b0 = int(cfg.boff[c])
                    nblk = M * cfg.ch_tiles[c]
                    for g in range(nblk * 128 // GCH):
                        gb = GCH // 128
                        i0 = (b0 + g * gb) * 128
                        nc.gpsimd.dma_gather(
                            Zb[:, b0 + g * gb:b0 + (g + 1) * gb, :],
                            table.ap(),
                            GIDX[:, i0 // 16:(i0 + GCH) // 16],
                            GCH, GCH, 128)

            def routing_iter(c, t, k):
                f = k * KD
                t0, T = cfg.tbase[c], cfg.ch_tiles[c]
                B = M * T
                b0 = int(cfg.boff[c])
                zq = Zb[:, b0:b0 + B, :f]
                zq_m = zq.rearrange("p (m j) f -> p m j f", m=M)
                zq_dk = zq.rearrange("p b (d k) -> p b d k", k=k)
                sq_ = S[:, :B * k].rearrange("p (b k) -> p b k", k=k)
                sq_flat = S[:, :B * k]
                sq4 = S[:, :B * k].rearrange("p (m j k) -> p m j k", m=M, k=k)
                PRb = PR[:, :B, :]
                PRm = PR[:, :B, :].rearrange("p (m j) f -> p m j f", m=M)
                den = DEN[:, :B]
                riv = RIV[:, :B]
                xsl = XN[:, t0:t0 + T, :f]
                usl = U2[:, t0:t0 + T, :f]
                usrc = xsl if t == 0 else usl
                # scores: PR = z * u ; tree-reduce over d -> S
                nc.vector.tensor_tensor(
                    out=PRm[:, :, :, :f], in0=zq_m,
                    in1=usrc.unsqueeze(1).broadcast_to([128, M, T, f]),
                    op=mybir.AluOpType.mult)
                cur = f
                while cur > k:
                    h = cur // 2
                    dst = PRb[:, :, :h] if h > k else sq_
                    nc.vector.tensor_tensor(
                        out=dst, in0=PRb[:, :, :h],
                        in1=PRb[:, :, h:cur], op=mybir.AluOpType.add)
                    cur = h
                if t > 0:
                    # u was left unnormalized; scale scores by 1/||u||
                    nc.vector.tensor_tensor(
                        out=sq4, in0=sq4,
                        in1=RIN2[:, :T * k].rearrange(
                            "p (j k) -> p j k", k=k).unsqueeze(1)
                            .broadcast_to([128, M, T, k]),
                        op=mybir.AluOpType.mult)
                # softmax over k (scores bounded; no max shift)
                nc.scalar.activation(sq_flat, sq_flat, AF.Exp)
                nc.vector.tensor_reduce(
                    out=den, in_=sq_,
                    op=mybir.AluOpType.add, axis=mybir.AxisListType.X)
                nc.vector.reciprocal_approx_fast(riv, den)
                nc.vector.tensor_tensor(
                    out=sq_, in0=sq_,
                    in1=riv.unsqueeze(2).broadcast_to([128, B, k]),
                    op=mybir.AluOpType.mult)
                # aggregate: PR = z * p ; tree over m ; + x_norm
                nc.vector.tensor_tensor(
                    out=PRb[:, :, :f].rearrange("p b (d k) -> p b d k", k=k),
                    in0=zq_dk,
                    in1=sq_.unsqueeze(2).broadcast_to([128, B, KD, k]),
                    op=mybir.AluOpType.mult)
                cm = M
                while cm > 1:
                    h = cm // 2
                    nc.vector.tensor_tensor(
                        out=PRm[:, :h, :, :f], in0=PRm[:, :h, :, :f],
                        in1=PRm[:, h:cm, :, :f], op=mybir.AluOpType.add)
                    cm = h
                nc.vector.tensor_tensor(
                    out=usl, in0=PRm[:, 0, :, :f], in1=xsl,
                    op=mybir.AluOpType.add)
                if t < cfg.routit - 1:
                    norm_stats(U2, c, k)
                else:
                    nc.scalar.activation(XC[:, t0:t0 + T, :f], usl, AF.Relu)

            NCH = len(cfg.ch_tiles)

            # ---- prologue: PCA + normalize + table + gathers for layer 0 ----
            for c in range(NCH):
                pca_chunk(c)
                normalize_chunk(XC, XN, c, cfg.caps[0])
                tshard_chunk(c)
                allgathers(c)
            gathers()
            nc.sync.dma_start(
                outs_d[0].ap().rearrange("(j p) f -> p j f", p=128), XC)

            # ---- routing layers -------------------------------------------
            for li, k in enumerate(cfg.caps):
                f = k * KD
                for c in range(NCH):
                    for t in range(cfg.routit):
                        routing_iter(c, t, k)
                    if li < len(cfg.caps) - 1:
                        linear_chunk(li + 1, c)
                        normalize_chunk(U2, XN, c, cfg.caps[li + 1])
                        tshard_chunk(c)
                        allgathers(c)
                if li < len(cfg.caps) - 1:
                    gathers()
                nc.sync.dma_start(
                    outs_d[li + 1].ap().rearrange("(j p) f -> p j f", p=128),
                    XC[:, :, :f])

    nc.compile()
    return nc


# ----------------------------------------------------------------------------
# Host-side prep / assembly
# ----------------------------------------------------------------------------

def prepare_in_maps(cfg: Cfg, feature, neighbor_id, pca_w, pca_b, ws, bs_):
    NS, NP = cfg.nshard, cfg.np_
    nb = np.asarray(neighbor_id).astype(np.int64)
    perms = [perm_dk(k) for k in cfg.caps]
    p0 = perms[0]

    pwt = np.zeros((cfg.fpad, 128), np.float16)
    pwt[:cfg.feat, :] = np.asarray(pca_w).T[:, p0].astype(np.float16)
    pbb = np.zeros((128, 1), np.float32)
    pbb[:, 0] = np.asarray(pca_b, np.float32)[p0]
    wts, bss = [], []
    for i, (w, b) in enumerate(zip(ws, bs_)):
        fo, fi = w.shape
        wp = np.asarray(w)[perms[i + 1]][:, perms[i]]   # out-perm, in-perm
        wt = np.zeros((128, fo), np.float16)
        wt[:fi, :] = wp.T.astype(np.float16)
        wts.append(wt)
        bb = np.zeros((128, 1), np.float32)
        bb[:fo, 0] = np.asarray(b, np.float32)[perms[i + 1]]
        bss.append(bb)

    rows_a = cfg.rows_a
    base_b = cfg.n_cores * rows_a

    def table_row(G):
        c, g = np.divmod(G, NS)
        j, p = np.divmod(g, 128)
        ra = c * rows_a + j * 128 + p
        rb = base_b + c * cfg.rows_b + (j - AG_SPLIT) * 128 + p
        return np.where(j < AG_SPLIT, ra, rb)

    in_maps = []
    for c in range(cfg.n_cores):
        lo = c * NS
        ft = np.zeros((cfg.fpad, NP), np.float16)
        ft[:cfg.feat, :NS] = np.asarray(feature[lo:lo + NS]).T.astype(np.float16)

        rows = np.zeros((NP, M), np.int64)
        rows[:NS] = table_row(nb[lo:lo + NS, :])
        # gather order: chunk-major, then m, then tile (jj), then p
        parts = []
        for ci in range(len(cfg.ch_tiles)):
            t0, T = int(cfg.tbase[ci]), cfg.ch_tiles[ci]
            r = rows[t0 * 128:(t0 + T) * 128, :].reshape(T * 128, M)
            parts.append(r.T.reshape(-1))            # [m, jj*128]
        gidx = np.concatenate(parts).astype(np.int16)
        gidx_w = np.tile(gidx.reshape(-1, 16).T, (8, 1))

        m = {"feat_t": ft, "pca_wT": pwt, "pca_b": pbb, "gidx": gidx_w}
        for i in range(len(wts)):
            m[f"w{i + 1}T"] = wts[i]
            m[f"b{i + 1}"] = bss[i]
        in_maps.append(m)
    return in_maps


def assemble_output(cfg: Cfg, results):
    NS = cfg.nshard
    fdims = [128] + [k * KD for k in cfg.caps]
    perms = [perm_dk(k) for k in [8] + list(cfg.caps)]
    cols = []
    for li, f in enumerate(fdims):
        perm = perms[li]
        shards = []
        for c in range(cfg.n_cores):
            a = np.asarray(results[c][f"y{li}"]).astype(np.float32)[:NS]
            u = np.empty_like(a)
            u[:, perm] = a                                # undo (d,k) packing
            shards.append(u)
        cols.append(np.concatenate(shards, axis=0))
    return np.concatenate(cols, axis=1)


def _ensure_ntff_hook():
    try:
        from antenv.axon_hooks import get_axon_ntff_profile_hook  # noqa: F401
        return True
    except ImportError:
        pass
    try:
        import types
        import antenv
        from trn_agent_boot.trn_boot import _ntff_profile_via_ctypes
        mod = types.ModuleType("antenv.axon_hooks")
        state = {"h": None}
        mod.set_axon_ntff_profile_hook = lambda h: state.__setitem__("h", h)
        mod.get_axon_ntff_profile_hook = lambda: state["h"]
        sys.modules["antenv.axon_hooks"] = mod
        antenv.axon_hooks = mod
        mod.set_axon_ntff_profile_hook(
            _ntff_profile_via_ctypes("/opt/axon/libaxon_pjrt.so"))
        return True
    except Exception:
        return False


_CACHE = {}


def _get_nc(cfg: Cfg):
    key = (cfg.nshard, cfg.feat, cfg.n_cores)
    if key not in _CACHE:
        _CACHE[key] = build_nc(cfg)
    return _CACHE[key]


def kernel(feature, neighbor_id, pca_w, pca_b,
           w1, b1, w2, b2, w3, b3, w4, b4, w5, b5):
    cfg = FULL_CFG
    nc = _get_nc(cfg)
    in_maps = prepare_in_maps(
        cfg, np.asarray(feature), np.asarray(neighbor_id),
        np.asarray(pca_w), np.asarray(pca_b),
        [np.asarray(w) for w in (w1, w2, w3, w4, w5)],
        [np.asarray(b) for b in (b1, b2, b3, b4, b5)])
    trace = bool(int(os.environ.get("KERNEL_TRACE", "0")))
    if trace:
        trace = _ensure_ntff_hook()
    tmpdir = os.environ.get("KERNEL_TRACE_DIR") or None
    res = run_bass_kernel_spmd(nc, in_maps, core_ids=list(range(cfg.n_cores)),
                               trace=trace, tmpdir=tmpdir)
    out = assemble_output(cfg, res.results)
    if trace:
        kernel.last_exec_time_ns = res.exec_time_ns
    return out


kernel.last_exec_time_ns = None


# revision 4
# speedup vs baseline: 1.1643x; 1.0995x over previous
"""DisenGCN Trainium2 kernel (8 NeuronCores, SPMD node-parallel).

Strategy (hardcoded from the problem spec):
  - Shard the 20000 nodes across 8 cores (2500/core, padded to 2560 = 20*128).
  - Weights replicated; per layer each core computes its local normalized
    embedding shard, AllGathers the full table to DRAM, then dma_gathers its
    neighbor rows into SBUF and runs the 5 capsule-routing iterations on the
    Vector/Scalar engines (node-major layout: nodes on partitions).
  - Features are stored in (d, k) transposed capsule order (host-side weight
    permutation) so per-capsule reductions are flat prefix-halving tree adds
    and all broadcast multiplies have contiguous innermost APs (DVE 2x mode).
  - Nodes are processed in 6 chunks of [2,4,4,4,4,2] tiles so the dma_gather
    descriptor generation (GPSIMD-bound, ~8us/1024 rows) of chunk c+1
    overlaps the routing DVE work of chunk c, and the per-layer tail
    (linear -> normalize -> tshard -> AllGather -> first gather) is short.
  - The AllGather is split in two pieces (tiles 0-17 / 18-19, j-major table
    rows) so the first piece overlaps the last chunks' routing.
  - 1/sqrt(ss) is computed as exp(-0.5*ln(ss+1e-12)) on the ACT engine: Ln,
    Exp, Square, Relu, Identity all live in one activation table set
    (natural_log_exp_and_others), so the inner loop never reloads tables.
  - fp16 storage/compute; fp32 only for softmax denominators and ln output.
"""

import os
import sys
import numpy as np

for _p in ("/opt/trn_rl_repo", "/root/.axon_site/_ro/trn_rl_repo"):
    if os.path.isdir(_p) and _p not in sys.path:
        sys.path.insert(0, _p)

import concourse.bass as bass  # noqa: E402
import concourse.tile as tile  # noqa: E402
from concourse import bacc, mybir  # noqa: E402
from concourse.bass_utils import run_bass_kernel_spmd  # noqa: E402
from concourse.library_config import mlp as mlp_lib  # noqa: E402
from concourse.masks import make_identity  # noqa: E402

FP16 = mybir.dt.float16
FP32 = mybir.dt.float32
I16 = mybir.dt.int16
AF = mybir.ActivationFunctionType

N_CORES = 8
M = 16          # neighbor fanout
KD = 16         # per-capsule dim
CAPS = [8, 7, 6, 5, 4, 3]
ROUTIT = 5
GCH = 1024      # dma_gather rows per instruction (>1024 overflows the ring)

CH_TILES = [2, 4, 4, 4, 4, 2]        # chunk sizes (tiles of 128 nodes)
AG_SPLIT = 18                        # tiles [0,18) = piece A, [18,20) = B


def perm_dk(k):
    """new position d*k+kk  <-  old feature index kk*KD+d."""
    p = np.empty(k * KD, np.int64)
    for d in range(KD):
        for kk in range(k):
            p[d * k + kk] = kk * KD + d
    return p


class Cfg:
    def __init__(self, nshard, feat, n_cores=N_CORES, caps=CAPS, routit=ROUTIT):
        self.n_cores = n_cores
        self.nshard = nshard
        self.np_ = ((nshard + 127) // 128) * 128
        self.nt = self.np_ // 128
        self.E = self.np_ * M
        self.feat = feat
        self.fpad = ((feat + 127) // 128) * 128
        self.fchunks = self.fpad // 128
        self.caps = caps
        self.routit = routit
        self.kmax = max(caps)
        self.ntab = n_cores * self.np_
        assert sum(CH_TILES) == self.nt
        self.ch_tiles = CH_TILES
        self.tbase = np.concatenate([[0], np.cumsum(CH_TILES)])[:-1]
        self.boff = np.concatenate([[0], np.cumsum([M * t for t in CH_TILES])])
        self.tmax = max(CH_TILES)
        # j-major table rows, split at AG_SPLIT tiles
        self.rows_a = AG_SPLIT * 128                 # per-core piece A rows
        self.rows_b = (self.nt - AG_SPLIT) * 128     # per-core piece B rows


FULL_CFG = Cfg(2500, 500)


def build_nc(cfg: Cfg):
    nc = bacc.Bacc("TRN2", target_bir_lowering=False, debug=False,
                   num_devices=cfg.n_cores)
    NT, NP, E = cfg.nt, cfg.np_, cfg.E
    KM = cfg.kmax
    TM = cfg.tmax
    BM = M * TM

    feat_t = nc.dram_tensor("feat_t", [cfg.fpad, NP], FP16, kind="ExternalInput")
    pca_wT = nc.dram_tensor("pca_wT", [cfg.fpad, 128], FP16, kind="ExternalInput")
    pca_b = nc.dram_tensor("pca_b", [128, 1], FP32, kind="ExternalInput")
    wTs, bs = [], []
    for i in range(1, len(cfg.caps)):
        fo = cfg.caps[i] * KD
        wTs.append(nc.dram_tensor(f"w{i}T", [128, fo], FP16, kind="ExternalInput"))
        bs.append(nc.dram_tensor(f"b{i}", [128, 1], FP32, kind="ExternalInput"))
    gidx_d = nc.dram_tensor("gidx", [128, E // 16], I16, kind="ExternalInput")
    outs_d = []
    fdims = [128] + [k * KD for k in cfg.caps]
    for li, f in enumerate(fdims):
        outs_d.append(nc.dram_tensor(f"y{li}", [NP, f], FP16, kind="ExternalOutput"))

    tshard = nc.dram_tensor("tshard", [NP, 128], FP16)
    table = nc.dram_tensor("table", [cfg.ntab, 128], FP16, addr_space="Shared")

    def sb(name, shape, dt):
        return nc.alloc_sbuf_tensor(name, shape, dt).ap()

    Z = sb("Z", [128, M * NT, 128], FP16)       # chunk-major [c][m][jj] blocks
    XC = sb("XC", [128, NT, 128], FP16)
    XN = sb("XN", [128, NT, 128], FP16)
    U2 = sb("U2", [128, NT, 128], FP16)
    XT = sb("XT", [128, NP], FP16)
    XLT = sb("XLT", [128, NP], FP16)
    PR = sb("PR", [128, BM, 128], FP16)         # per-chunk scratch
    S = sb("S", [128, BM * KM], FP16)           # scores -> exp -> p (in place)
    DEN = sb("DEN", [128, BM], FP32)
    RIV = sb("RIV", [128, BM], FP32)
    SQ = sb("SQ", [128, TM, 128], FP16)         # normalize scratch
    RIN = sb("RIN", [128, TM * KM], FP16)
    LNS = sb("LNS", [128, TM * KM], FP32)
    RIN2 = sb("RIN2", [128, TM * KM], FP16)
    EPS = sb("EPS", [128, 1], FP32)
    GIDX = sb("GIDX", [128, E // 16], I16)
    IDT = sb("IDT", [128, 128], FP16)
    FT = sb("FT", [128, cfg.fchunks, NP], FP16)
    PW = sb("PW", [128, cfg.fchunks, 128], FP16)
    PB = sb("PB", [128, 1], FP32)
    WTS = [sb(f"WTS{i}", [128, cfg.caps[i] * KD], FP16)
           for i in range(1, len(cfg.caps))]
    BS = [sb(f"BS{i}", [128, 1], FP32) for i in range(1, len(cfg.caps))]

    Zb = Z.rearrange("p (b f) -> p b f", f=128) if Z.ndim == 2 else Z

    with tile.TileContext(nc) as tc:
        import contextlib
        ctx = contextlib.ExitStack()
        with ctx:
            psum = ctx.enter_context(tc.tile_pool(name="psum", bufs=2, space="PSUM"))
            nc.gpsimd.load_library(mlp_lib)
            make_identity(nc, IDT)
            nc.vector.memset(EPS, 1e-12)

            nc.sync.dma_start(GIDX, gidx_d.ap())
            nc.sync.dma_start(FT, feat_t.ap().rearrange("(c p) n -> p c n", p=128))
            nc.sync.dma_start(PW, pca_wT.ap().rearrange("(c p) f -> p c f", p=128))
            nc.sync.dma_start(PB, pca_b.ap())
            for i in range(len(cfg.caps) - 1):
                nc.sync.dma_start(WTS[i], wTs[i].ap())
                nc.sync.dma_start(BS[i], bs[i].ap())

            def transpose_block(dst_ap, src_ap, fin, fout):
                pt = psum.tile([128, 128], FP16, tag="pt")
                nc.tensor.transpose(pt[:fout, :fin], src_ap, IDT[:fin, :fin])
                nc.scalar.copy(dst_ap, pt[:fout, :fin])

            def pca_chunk(c):
                t0, T = cfg.tbase[c], cfg.ch_tiles[c]
                cs, ce = t0 * 128, (t0 + T) * 128
                pl = psum.tile([128, TM * 128], FP32, tag="pl")
                for q in range(cfg.fchunks):
                    nc.tensor.matmul(
                        pl[:, :T * 128], PW[:, q, :], FT[:, q, cs:ce],
                        start=(q == 0), stop=(q == cfg.fchunks - 1))
                nc.scalar.activation(XLT[:, cs:ce], pl[:, :T * 128],
                                     AF.Relu, bias=PB[:, :], scale=1.0)
                for j in range(t0, t0 + T):
                    transpose_block(XC[:, j, :], XLT[:, j * 128:(j + 1) * 128],
                                    128, 128)

            def linear_chunk(li, c):
                # target layer li (1..5): XC (fin) -> U2 (fout)
                fin = cfg.caps[li - 1] * KD
                fout = cfg.caps[li] * KD
                t0, T = cfg.tbase[c], cfg.ch_tiles[c]
                cs, ce = t0 * 128, (t0 + T) * 128
                for j in range(t0, t0 + T):
                    transpose_block(XT[:fin, j * 128:(j + 1) * 128],
                                    XC[:, j, :fin], 128, fin)
                pl = psum.tile([128, TM * 128], FP32, tag="pl")
                nc.tensor.matmul(pl[:fout, :T * 128], WTS[li - 1][:fin, :fout],
                                 XT[:fin, cs:ce], start=True, stop=True)
                nc.scalar.activation(XLT[:fout, cs:ce], pl[:fout, :T * 128],
                                     AF.Identity, bias=BS[li - 1][:fout, :],
                                     scale=1.0)
                for j in range(t0, t0 + T):
                    transpose_block(U2[:, j, :fout], XLT[:fout, j * 128:(j + 1) * 128],
                                    fout, 128)

            def norm_stats(src, c, k):
                """RIN2 = 1/||src_chunk|| per capsule: exp(-0.5*ln(ss+eps))."""
                f = k * KD
                t0, T = cfg.tbase[c], cfg.ch_tiles[c]
                nc.scalar.activation(SQ[:, :T, :f], src[:, t0:t0 + T, :f],
                                     AF.Square)
                cur = f
                rin = RIN[:, :T * k]
                while cur > k:
                    h = cur // 2
                    if h > k:
                        nc.vector.tensor_tensor(
                            out=SQ[:, :T, :h], in0=SQ[:, :T, :h],
                            in1=SQ[:, :T, h:cur], op=mybir.AluOpType.add)
                    else:
                        nc.vector.tensor_tensor(
                            out=rin.rearrange("p (j k) -> p j k", k=k),
                            in0=SQ[:, :T, :h], in1=SQ[:, :T, h:cur],
                            op=mybir.AluOpType.add)
                    cur = h
                nc.scalar.activation(LNS[:, :T * k], rin, AF.Ln,
                                     bias=EPS[:, :], scale=1.0)
                nc.scalar.activation(RIN2[:, :T * k], LNS[:, :T * k],
                                     AF.Exp, scale=-0.5)

            def normalize_chunk(src, dst, c, k):
                """dst = per-capsule l2 normalize of src for chunk c."""
                f = k * KD
                t0, T = cfg.tbase[c], cfg.ch_tiles[c]
                norm_stats(src, c, k)
                nc.vector.tensor_tensor(
                    out=dst[:, t0:t0 + T, :f].rearrange(
                        "p j (d k) -> p j d k", k=k),
                    in0=src[:, t0:t0 + T, :f].rearrange(
                        "p j (d k) -> p j d k", k=k),
                    in1=RIN2[:, :T * k].rearrange("p (j k) -> p j k", k=k)
                        .unsqueeze(2).broadcast_to([128, T, KD, k]),
                    op=mybir.AluOpType.mult)

            def tshard_chunk(c):
                t0, T = cfg.tbase[c], cfg.ch_tiles[c]
                nc.sync.dma_start(
                    tshard.ap()[t0 * 128:(t0 + T) * 128, :]
                    .rearrange("(j p) f -> p j f", p=128),
                    XN[:, t0:t0 + T, :])

            def allgathers(c):
                if cfg.tbase[c] + cfg.ch_tiles[c] == AG_SPLIT:
                    nc.gpsimd.collective_compute(
                        "AllGather", mybir.AluOpType.bypass,
                        replica_groups=[list(range(cfg.n_cores))],
                        ins=[tshard.ap()[0:cfg.rows_a, :]],
                        outs=[table.ap()[0:cfg.n_cores * cfg.rows_a, :]])
                if cfg.tbase[c] + cfg.ch_tiles[c] == NT:
                    nc.gpsimd.collective_compute(
                        "AllGather", mybir.AluOpType.bypass,
                        replica_groups=[list(range(cfg.n_cores))],
                        ins=[tshard.ap()[cfg.rows_a:NP, :]],
                        outs=[table.ap()[cfg.n_cores * cfg.rows_a:cfg.ntab, :]])

            def gathers():
                for c in range(len(cfg.ch_tiles)):
                    b0 = int(cfg.boff[c])
                    nblk = M * cfg.ch_tiles[c]
                    for g in range(nblk * 128 // GCH):
                        gb = GCH // 128
                        i0 = (b0 + g * gb) * 128
                        nc.gpsimd.dma_gather(
                            Zb[:, b0 + g * gb:b0 + (g + 1) * gb, :],
                            table.ap(),
                            GIDX[:, i0 // 16:(i0 + GCH) // 16],
                            GCH, GCH, 128)

            def routing_iter(c, t, k):
                f = k * KD
                t0, T = cfg.tbase[c], cfg.ch_tiles[c]
                B = M * T
                b0 = int(cfg.boff[c])
                zq = Zb[:, b0:b0 + B, :f]
                zq_m = zq.rearrange("p (m j) f -> p m j f", m=M)
                zq_dk = zq.rearrange("p b (d k) -> p b d k", k=k)
                sq_ = S[:, :B * k].rearrange("p (b k) -> p b k", k=k)
                sq_flat = S[:, :B * k]
                sq4 = S[:, :B * k].rearrange("p (m j k) -> p m j k", m=M, k=k)
                PRb = PR[:, :B, :]
                PRm = PR[:, :B, :].rearrange("p (m j) f -> p m j f", m=M)
                den = DEN[:, :B]
                riv = RIV[:, :B]
                xsl = XN[:, t0:t0 + T, :f]
                usl = U2[:, t0:t0 + T, :f]
                usrc = xsl if t == 0 else usl
                # scores: PR = z * u ; tree-reduce over d -> S
                nc.vector.tensor_tensor(
                    out=PRm[:, :, :, :f], in0=zq_m,
                    in1=usrc.unsqueeze(1).broadcast_to([128, M, T, f]),
                    op=mybir.AluOpType.mult)
                cur = f
                while cur > k:
                    h = cur // 2
                    dst = PRb[:, :, :h] if h > k else sq_
                    nc.vector.tensor_tensor(
                        out=dst, in0=PRb[:, :, :h],
                        in1=PRb[:, :, h:cur], op=mybir.AluOpType.add)
                    cur = h
                if t > 0:
                    # u was left unnormalized; scale scores by 1/||u||
                    nc.vector.tensor_tensor(
                        out=sq4, in0=sq4,
                        in1=RIN2[:, :T * k].rearrange(
                            "p (j k) -> p j k", k=k).unsqueeze(1)
                            .broadcast_to([128, M, T, k]),
                        op=mybir.AluOpType.mult)
                # softmax over k (scores bounded; no max shift)
                nc.scalar.activation(sq_flat, sq_flat, AF.Exp)
                nc.vector.tensor_reduce(
                    out=den, in_=sq_,
                    op=mybir.AluOpType.add, axis=mybir.AxisListType.X)
                nc.vector.reciprocal_approx_fast(riv, den)
                nc.vector.tensor_tensor(
                    out=sq_, in0=sq_,
                    in1=riv.unsqueeze(2).broadcast_to([128, B, k]),
                    op=mybir.AluOpType.mult)
                # aggregate: PR = z * p ; tree over m ; + x_norm
                nc.vector.tensor_tensor(
                    out=PRb[:, :, :f].rearrange("p b (d k) -> p b d k", k=k),
                    in0=zq_dk,
                    in1=sq_.unsqueeze(2).broadcast_to([128, B, KD, k]),
                    op=mybir.AluOpType.mult)
                cm = M
                while cm > 1:
                    h = cm // 2
                    nc.vector.tensor_tensor(
                        out=PRm[:, :h, :, :f], in0=PRm[:, :h, :, :f],
                        in1=PRm[:, h:cm, :, :f], op=mybir.AluOpType.add)
                    cm = h
                nc.vector.tensor_tensor(
                    out=usl, in0=PRm[:, 0, :, :f], in1=xsl,
                    op=mybir.AluOpType.add)
                if t < cfg.routit - 1:
                    norm_stats(U2, c, k)
                else:
                    nc.scalar.activation(XC[:, t0:t0 + T, :f], usl, AF.Relu)

            NCH = len(cfg.ch_tiles)

            # ---- prologue: PCA + normalize + table + gathers for layer 0 ----
            for c in range(NCH):
                pca_chunk(c)
                normalize_chunk(XC, XN, c, cfg.caps[0])
                tshard_chunk(c)
                allgathers(c)
            gathers()
            nc.sync.dma_start(
                outs_d[0].ap().rearrange("(j p) f -> p j f", p=128), XC)

            # ---- routing layers -------------------------------------------
            for li, k in enumerate(cfg.caps):
                f = k * KD
                for c in range(NCH):
                    for t in range(cfg.routit):
                        routing_iter(c, t, k)
                    if li < len(cfg.caps) - 1:
                        linear_chunk(li + 1, c)
                        normalize_chunk(U2, XN, c, cfg.caps[li + 1])
                        tshard_chunk(c)
                        allgathers(c)
                if li < len(cfg.caps) - 1:
                    gathers()
                nc.sync.dma_start(
                    outs_d[li + 1].ap().rearrange("(j p) f -> p j f", p=128),
                    XC[:, :, :f])

    nc.compile()
    return nc


# ----------------------------------------------------------------------------
# Host-side prep / assembly
# ----------------------------------------------------------------------------

def prepare_in_maps(cfg: Cfg, feature, neighbor_id, pca_w, pca_b, ws, bs_):
    NS, NP = cfg.nshard, cfg.np_
    nb = np.asarray(neighbor_id).astype(np.int64)
    perms = [perm_dk(k) for k in cfg.caps]
    p0 = perms[0]

    pwt = np.zeros((cfg.fpad, 128), np.float16)
    pwt[:cfg.feat, :] = np.asarray(pca_w).T[:, p0].astype(np.float16)
    pbb = np.zeros((128, 1), np.float32)
    pbb[:, 0] = np.asarray(pca_b, np.float32)[p0]
    wts, bss = [], []
    for i, (w, b) in enumerate(zip(ws, bs_)):
        fo, fi = w.shape
        wp = np.asarray(w)[perms[i + 1]][:, perms[i]]   # out-perm, in-perm
        wt = np.zeros((128, fo), np.float16)
        wt[:fi, :] = wp.T.astype(np.float16)
        wts.append(wt)
        bb = np.zeros((128, 1), np.float32)
        bb[:fo, 0] = np.asarray(b, np.float32)[perms[i + 1]]
        bss.append(bb)

    rows_a = cfg.rows_a
    base_b = cfg.n_cores * rows_a

    def table_row(G):
        c, g = np.divmod(G, NS)
        j, p = np.divmod(g, 128)
        ra = c * rows_a + j * 128 + p
        rb = base_b + c * cfg.rows_b + (j - AG_SPLIT) * 128 + p
        return np.where(j < AG_SPLIT, ra, rb)

    in_maps = []
    for c in range(cfg.n_cores):
        lo = c * NS
        ft = np.zeros((cfg.fpad, NP), np.float16)
        ft[:cfg.feat, :NS] = np.asarray(feature[lo:lo + NS]).T.astype(np.float16)

        rows = np.zeros((NP, M), np.int64)
        rows[:NS] = table_row(nb[lo:lo + NS, :])
        # gather order: chunk-major, then m, then tile (jj), then p
        parts = []
        for ci in range(len(cfg.ch_tiles)):
            t0, T = int(cfg.tbase[ci]), cfg.ch_tiles[ci]
            r = rows[t0 * 128:(t0 + T) * 128, :].reshape(T * 128, M)
            parts.append(r.T.reshape(-1))            # [m, jj*128]
        gidx = np.concatenate(parts).astype(np.int16)
        gidx_w = np.tile(gidx.reshape(-1, 16).T, (8, 1))

        m = {"feat_t": ft, "pca_wT": pwt, "pca_b": pbb, "gidx": gidx_w}
        for i in range(len(wts)):
            m[f"w{i + 1}T"] = wts[i]
            m[f"b{i + 1}"] = bss[i]
        in_maps.append(m)
    return in_maps


def assemble_output(cfg: Cfg, results):
    NS = cfg.nshard
    fdims = [128] + [k * KD for k in cfg.caps]
    perms = [perm_dk(k) for k in [8] + list(cfg.caps)]
    cols = []
    for li, f in enumerate(fdims):
        perm = perms[li]
        shards = []
        for c in range(cfg.n_cores):
            a = np.asarray(results[c][f"y{li}"]).astype(np.float32)[:NS]
            u = np.empty_like(a)
            u[:, perm] = a                                # undo (d,k) packing
            shards.append(u)
        cols.append(np.concatenate(shards, axis=0))
    return np.concatenate(cols, axis=1)


def _ensure_ntff_hook():
    try:
        from antenv.axon_hooks import get_axon_ntff_profile_hook  # noqa: F401
        return True
    except ImportError:
        pass
    try:
        import types
        import antenv
        from trn_agent_boot.trn_boot import _ntff_profile_via_ctypes
        mod = types.ModuleType("antenv.axon_hooks")
        state = {"h": None}
        mod.set_axon_ntff_profile_hook = lambda h: state.__setitem__("h", h)
        mod.get_axon_ntff_profile_hook = lambda: state["h"]
        sys.modules["antenv.axon_hooks"] = mod
        antenv.axon_hooks = mod
        mod.set_axon_ntff_profile_hook(
            _ntff_profile_via_ctypes("/opt/axon/libaxon_pjrt.so"))
        return True
    except Exception:
        return False


_CACHE = {}


def _build_with_unified_act_tables(cfg: Cfg):
    """Compile with Exp/Ln visible only in natural_log_exp_and_others.

    The act-table-load pass greedily picks the first set containing each
    function, which splits Exp (exp_and_others) from Ln (natural_log) and
    reloads tables every routing iteration.  Hiding Exp/Ln from the other
    sets (order and ids unchanged -- the id indexes the canonical list)
    makes it pick the one set that has both, so the inner loop runs with
    zero table reloads.  Restored right after compile.
    """
    orig = bacc.get_activation_tables
    target = "natural_log_exp_and_others"

    def patched(arch):
        tabs = orig(arch)
        if target in tabs:
            for name, fns in tabs.items():
                if name != target:
                    fns.discard(AF.Exp)
                    fns.discard(AF.Ln)
        return tabs

    bacc.get_activation_tables = patched
    try:
        return build_nc(cfg)
    finally:
        bacc.get_activation_tables = orig


def _get_nc(cfg: Cfg):
    key = (cfg.nshard, cfg.feat, cfg.n_cores)
    if key not in _CACHE:
        _CACHE[key] = _build_with_unified_act_tables(cfg)
    return _CACHE[key]


def kernel(feature, neighbor_id, pca_w, pca_b,
           w1, b1, w2, b2, w3, b3, w4, b4, w5, b5):
    cfg = FULL_CFG
    nc = _get_nc(cfg)
    in_maps = prepare_in_maps(
        cfg, np.asarray(feature), np.asarray(neighbor_id),
        np.asarray(pca_w), np.asarray(pca_b),
        [np.asarray(w) for w in (w1, w2, w3, w4, w5)],
        [np.asarray(b) for b in (b1, b2, b3, b4, b5)])
    trace = bool(int(os.environ.get("KERNEL_TRACE", "0")))
    if trace:
        trace = _ensure_ntff_hook()
    tmpdir = os.environ.get("KERNEL_TRACE_DIR") or None
    res = run_bass_kernel_spmd(nc, in_maps, core_ids=list(range(cfg.n_cores)),
                               trace=trace, tmpdir=tmpdir)
    out = assemble_output(cfg, res.results)
    if trace:
        kernel.last_exec_time_ns = res.exec_time_ns
    return out


kernel.last_exec_time_ns = None


# revision 5
# speedup vs baseline: 1.1894x; 1.0216x over previous
"""DisenGCN Trainium2 kernel (8 NeuronCores, SPMD node-parallel).

Strategy (hardcoded from the problem spec):
  - Shard the 20000 nodes across 8 cores (2500/core, padded to 2560 = 20*128).
  - Weights replicated; per layer each core computes its local normalized
    embedding shard, AllGathers the full table to DRAM, then dma_gathers its
    neighbor rows into SBUF and runs the 5 capsule-routing iterations on the
    Vector/Scalar engines (node-major layout: nodes on partitions).
  - Features are stored in (d, k) transposed capsule order (host-side weight
    permutation) so per-capsule reductions are flat prefix-halving tree adds
    and all broadcast multiplies have contiguous innermost APs (DVE 2x mode).
  - Nodes are processed in 6 chunks of [2,4,4,4,4,2] tiles so the dma_gather
    descriptor generation (GPSIMD-bound, ~8us/1024 rows) of chunk c+1
    overlaps the routing DVE work of chunk c, and the per-layer tail
    (linear -> normalize -> tshard -> AllGather -> first gather) is short.
  - The AllGather is split in two pieces (tiles 0-17 / 18-19, j-major table
    rows) so the first piece overlaps the last chunks' routing.
  - 1/sqrt(ss) is computed as exp(-0.5*ln(ss+1e-12)) on the ACT engine: Ln,
    Exp, Square, Relu, Identity all live in one activation table set
    (natural_log_exp_and_others), so the inner loop never reloads tables.
  - fp16 storage/compute; fp32 only for softmax denominators and ln output.
"""

import os
import sys
import numpy as np

for _p in ("/opt/trn_rl_repo", "/root/.axon_site/_ro/trn_rl_repo"):
    if os.path.isdir(_p) and _p not in sys.path:
        sys.path.insert(0, _p)

import concourse.bass as bass  # noqa: E402
import concourse.tile as tile  # noqa: E402
from concourse import bacc, mybir  # noqa: E402
from concourse.bass_utils import run_bass_kernel_spmd  # noqa: E402
from concourse.library_config import mlp as mlp_lib  # noqa: E402
from concourse.masks import make_identity  # noqa: E402

FP16 = mybir.dt.float16
FP32 = mybir.dt.float32
I16 = mybir.dt.int16
AF = mybir.ActivationFunctionType

N_CORES = 8
M = 16          # neighbor fanout
KD = 16         # per-capsule dim
CAPS = [8, 7, 6, 5, 4, 3]
ROUTIT = 5
GCH = 1024      # dma_gather rows per instruction (>1024 overflows the ring)

CH_TILES = [1, 3, 4, 4, 4, 4]        # chunk sizes (tiles of 128 nodes)
AG_SPLIT = 16                        # tiles [0,16) = piece A, [16,20) = B


def perm_dk(k):
    """new position d*k+kk  <-  old feature index kk*KD+d."""
    p = np.empty(k * KD, np.int64)
    for d in range(KD):
        for kk in range(k):
            p[d * k + kk] = kk * KD + d
    return p


class Cfg:
    def __init__(self, nshard, feat, n_cores=N_CORES, caps=CAPS, routit=ROUTIT):
        self.n_cores = n_cores
        self.nshard = nshard
        self.np_ = ((nshard + 127) // 128) * 128
        self.nt = self.np_ // 128
        self.E = self.np_ * M
        self.feat = feat
        self.fpad = ((feat + 127) // 128) * 128
        self.fchunks = self.fpad // 128
        self.caps = caps
        self.routit = routit
        self.kmax = max(caps)
        self.ntab = n_cores * self.np_
        assert sum(CH_TILES) == self.nt
        self.ch_tiles = CH_TILES
        self.tbase = np.concatenate([[0], np.cumsum(CH_TILES)])[:-1]
        self.boff = np.concatenate([[0], np.cumsum([M * t for t in CH_TILES])])
        self.tmax = max(CH_TILES)
        # j-major table rows, split at AG_SPLIT tiles
        self.rows_a = AG_SPLIT * 128                 # per-core piece A rows
        self.rows_b = (self.nt - AG_SPLIT) * 128     # per-core piece B rows
        self.base_b = n_cores * self.rows_a          # first piece-B table row
        # nA[c]: leading 1024-idx gather groups of chunk c whose indices all
        # fall in piece A (host-verified); those can gather right after AG-A.
        self.nA = [0] * len(CH_TILES)


FULL_CFG = Cfg(2500, 500)


def build_nc(cfg: Cfg):
    nc = bacc.Bacc("TRN2", target_bir_lowering=False, debug=False,
                   num_devices=cfg.n_cores)
    NT, NP, E = cfg.nt, cfg.np_, cfg.E
    KM = cfg.kmax
    TM = cfg.tmax
    BM = M * TM

    feat_t = nc.dram_tensor("feat_t", [cfg.fpad, NP], FP16, kind="ExternalInput")
    pca_wT = nc.dram_tensor("pca_wT", [cfg.fpad, 128], FP16, kind="ExternalInput")
    pca_b = nc.dram_tensor("pca_b", [128, 1], FP32, kind="ExternalInput")
    wTs, bs = [], []
    for i in range(1, len(cfg.caps)):
        fo = cfg.caps[i] * KD
        wTs.append(nc.dram_tensor(f"w{i}T", [128, fo], FP16, kind="ExternalInput"))
        bs.append(nc.dram_tensor(f"b{i}", [128, 1], FP32, kind="ExternalInput"))
    gidx_d = nc.dram_tensor("gidx", [128, E // 16], I16, kind="ExternalInput")
    outs_d = []
    fdims = [128] + [k * KD for k in cfg.caps]
    for li, f in enumerate(fdims):
        outs_d.append(nc.dram_tensor(f"y{li}", [NP, f], FP16, kind="ExternalOutput"))

    tshard = nc.dram_tensor("tshard", [NP, 128], FP16)
    table = nc.dram_tensor("table", [cfg.ntab, 128], FP16, addr_space="Shared")

    def sb(name, shape, dt):
        return nc.alloc_sbuf_tensor(name, shape, dt).ap()

    Z = sb("Z", [128, M * NT, 128], FP16)       # chunk-major [c][m][jj] blocks
    XC = sb("XC", [128, NT, 128], FP16)
    XN = sb("XN", [128, NT, 128], FP16)
    U2 = sb("U2", [128, NT, 128], FP16)
    XT = sb("XT", [128, NP], FP16)
    XLT = sb("XLT", [128, NP], FP16)
    PR = sb("PR", [128, BM, 128], FP16)         # per-chunk scratch
    S = sb("S", [128, BM * KM], FP16)           # scores -> exp -> p (in place)
    DEN = sb("DEN", [128, BM], FP32)
    RIV = sb("RIV", [128, BM], FP32)
    SQ = sb("SQ", [128, TM, 128], FP16)         # normalize scratch
    RIN = sb("RIN", [128, TM * KM], FP16)
    LNS = sb("LNS", [128, TM * KM], FP32)
    RIN2 = sb("RIN2", [128, TM * KM], FP16)
    EPS = sb("EPS", [128, 1], FP32)
    GIDX = sb("GIDX", [128, E // 16], I16)
    IDT = sb("IDT", [128, 128], FP16)
    FT = sb("FT", [128, cfg.fchunks, NP], FP16)
    PW = sb("PW", [128, cfg.fchunks, 128], FP16)
    PB = sb("PB", [128, 1], FP32)
    WTS = [sb(f"WTS{i}", [128, cfg.caps[i] * KD], FP16)
           for i in range(1, len(cfg.caps))]
    BS = [sb(f"BS{i}", [128, 1], FP32) for i in range(1, len(cfg.caps))]

    Zb = Z.rearrange("p (b f) -> p b f", f=128) if Z.ndim == 2 else Z

    with tile.TileContext(nc) as tc:
        import contextlib
        ctx = contextlib.ExitStack()
        with ctx:
            psum = ctx.enter_context(tc.tile_pool(name="psum", bufs=2, space="PSUM"))
            nc.gpsimd.load_library(mlp_lib)
            make_identity(nc, IDT)
            nc.vector.memset(EPS, 1e-12)

            nc.sync.dma_start(GIDX, gidx_d.ap())
            nc.sync.dma_start(FT, feat_t.ap().rearrange("(c p) n -> p c n", p=128))
            nc.sync.dma_start(PW, pca_wT.ap().rearrange("(c p) f -> p c f", p=128))
            nc.sync.dma_start(PB, pca_b.ap())
            for i in range(len(cfg.caps) - 1):
                nc.sync.dma_start(WTS[i], wTs[i].ap())
                nc.sync.dma_start(BS[i], bs[i].ap())

            def transpose_block(dst_ap, src_ap, fin, fout):
                pt = psum.tile([128, 128], FP16, tag="pt")
                nc.tensor.transpose(pt[:fout, :fin], src_ap, IDT[:fin, :fin])
                nc.scalar.copy(dst_ap, pt[:fout, :fin])

            def pca_chunk(c):
                t0, T = cfg.tbase[c], cfg.ch_tiles[c]
                cs, ce = t0 * 128, (t0 + T) * 128
                pl = psum.tile([128, TM * 128], FP32, tag="pl")
                for q in range(cfg.fchunks):
                    nc.tensor.matmul(
                        pl[:, :T * 128], PW[:, q, :], FT[:, q, cs:ce],
                        start=(q == 0), stop=(q == cfg.fchunks - 1))
                nc.scalar.activation(XLT[:, cs:ce], pl[:, :T * 128],
                                     AF.Relu, bias=PB[:, :], scale=1.0)
                for j in range(t0, t0 + T):
                    transpose_block(XC[:, j, :], XLT[:, j * 128:(j + 1) * 128],
                                    128, 128)

            def linear_chunk(li, c):
                # target layer li (1..5): XC (fin) -> U2 (fout)
                fin = cfg.caps[li - 1] * KD
                fout = cfg.caps[li] * KD
                t0, T = cfg.tbase[c], cfg.ch_tiles[c]
                cs, ce = t0 * 128, (t0 + T) * 128
                for j in range(t0, t0 + T):
                    transpose_block(XT[:fin, j * 128:(j + 1) * 128],
                                    XC[:, j, :fin], 128, fin)
                pl = psum.tile([128, TM * 128], FP32, tag="pl")
                nc.tensor.matmul(pl[:fout, :T * 128], WTS[li - 1][:fin, :fout],
                                 XT[:fin, cs:ce], start=True, stop=True)
                nc.scalar.activation(XLT[:fout, cs:ce], pl[:fout, :T * 128],
                                     AF.Identity, bias=BS[li - 1][:fout, :],
                                     scale=1.0)
                for j in range(t0, t0 + T):
                    transpose_block(U2[:, j, :fout], XLT[:fout, j * 128:(j + 1) * 128],
                                    fout, 128)

            def norm_stats(src, c, k):
                """RIN2 = 1/||src_chunk|| per capsule: exp(-0.5*ln(ss+eps))."""
                f = k * KD
                t0, T = cfg.tbase[c], cfg.ch_tiles[c]
                nc.scalar.activation(SQ[:, :T, :f], src[:, t0:t0 + T, :f],
                                     AF.Square)
                cur = f
                rin = RIN[:, :T * k]
                while cur > k:
                    h = cur // 2
                    if h > k:
                        nc.vector.tensor_tensor(
                            out=SQ[:, :T, :h], in0=SQ[:, :T, :h],
                            in1=SQ[:, :T, h:cur], op=mybir.AluOpType.add)
                    else:
                        nc.vector.tensor_tensor(
                            out=rin.rearrange("p (j k) -> p j k", k=k),
                            in0=SQ[:, :T, :h], in1=SQ[:, :T, h:cur],
                            op=mybir.AluOpType.add)
                    cur = h
                nc.scalar.activation(LNS[:, :T * k], rin, AF.Ln,
                                     bias=EPS[:, :], scale=1.0)
                nc.scalar.activation(RIN2[:, :T * k], LNS[:, :T * k],
                                     AF.Exp, scale=-0.5)

            def normalize_chunk(src, dst, c, k):
                """dst = per-capsule l2 normalize of src for chunk c."""
                f = k * KD
                t0, T = cfg.tbase[c], cfg.ch_tiles[c]
                norm_stats(src, c, k)
                nc.vector.tensor_tensor(
                    out=dst[:, t0:t0 + T, :f].rearrange(
                        "p j (d k) -> p j d k", k=k),
                    in0=src[:, t0:t0 + T, :f].rearrange(
                        "p j (d k) -> p j d k", k=k),
                    in1=RIN2[:, :T * k].rearrange("p (j k) -> p j k", k=k)
                        .unsqueeze(2).broadcast_to([128, T, KD, k]),
                    op=mybir.AluOpType.mult)

            def tshard_chunk(c):
                t0, T = cfg.tbase[c], cfg.ch_tiles[c]
                nc.sync.dma_start(
                    tshard.ap()[t0 * 128:(t0 + T) * 128, :]
                    .rearrange("(j p) f -> p j f", p=128),
                    XN[:, t0:t0 + T, :])

            def allgathers(c):
                if cfg.tbase[c] + cfg.ch_tiles[c] == AG_SPLIT:
                    nc.gpsimd.collective_compute(
                        "AllGather", mybir.AluOpType.bypass,
                        replica_groups=[list(range(cfg.n_cores))],
                        ins=[tshard.ap()[0:cfg.rows_a, :]],
                        outs=[table.ap()[0:cfg.n_cores * cfg.rows_a, :]])
                if cfg.tbase[c] + cfg.ch_tiles[c] == NT:
                    nc.gpsimd.collective_compute(
                        "AllGather", mybir.AluOpType.bypass,
                        replica_groups=[list(range(cfg.n_cores))],
                        ins=[tshard.ap()[cfg.rows_a:NP, :]],
                        outs=[table.ap()[cfg.n_cores * cfg.rows_a:cfg.ntab, :]])

            def gather_group(c, g, a_only):
                b0 = int(cfg.boff[c])
                gb = GCH // 128
                i0 = (b0 + g * gb) * 128
                src_ap = (table.ap()[0:cfg.base_b, :] if a_only
                          else table.ap())
                nc.gpsimd.dma_gather(
                    Zb[:, b0 + g * gb:b0 + (g + 1) * gb, :],
                    src_ap,
                    GIDX[:, i0 // 16:(i0 + GCH) // 16],
                    GCH, GCH, 128)

            EARLY_CH = (0, 1)

            def gathers_early():
                # A-only prefix groups of the first chunks: depend only on
                # AG piece A, so their descriptor generation overlaps the
                # last chunk's routing + AG piece B.
                for c in EARLY_CH:
                    for g in range(cfg.nA[c]):
                        gather_group(c, g, True)

            def gathers_rest():
                for c in range(len(cfg.ch_tiles)):
                    ng = M * cfg.ch_tiles[c] * 128 // GCH
                    g0 = cfg.nA[c] if c in EARLY_CH else 0
                    for g in range(g0, ng):
                        gather_group(c, g, False)

            def routing_iter(c, t, k):
                f = k * KD
                t0, T = cfg.tbase[c], cfg.ch_tiles[c]
                B = M * T
                b0 = int(cfg.boff[c])
                zq = Zb[:, b0:b0 + B, :f]
                zq_m = zq.rearrange("p (m j) f -> p m j f", m=M)
                zq_dk = zq.rearrange("p b (d k) -> p b d k", k=k)
                sq_ = S[:, :B * k].rearrange("p (b k) -> p b k", k=k)
                sq_flat = S[:, :B * k]
                sq4 = S[:, :B * k].rearrange("p (m j k) -> p m j k", m=M, k=k)
                PRb = PR[:, :B, :]
                PRm = PR[:, :B, :].rearrange("p (m j) f -> p m j f", m=M)
                den = DEN[:, :B]
                riv = RIV[:, :B]
                xsl = XN[:, t0:t0 + T, :f]
                usl = U2[:, t0:t0 + T, :f]
                usrc = xsl if t == 0 else usl
                # scores: PR = z * u ; tree-reduce over d -> S
                nc.vector.tensor_tensor(
                    out=PRm[:, :, :, :f], in0=zq_m,
                    in1=usrc.unsqueeze(1).broadcast_to([128, M, T, f]),
                    op=mybir.AluOpType.mult)
                cur = f
                while cur > k:
                    h = cur // 2
                    dst = PRb[:, :, :h] if h > k else sq_
                    nc.vector.tensor_tensor(
                        out=dst, in0=PRb[:, :, :h],
                        in1=PRb[:, :, h:cur], op=mybir.AluOpType.add)
                    cur = h
                if t > 0:
                    # u was left unnormalized; scale scores by 1/||u||
                    nc.vector.tensor_tensor(
                        out=sq4, in0=sq4,
                        in1=RIN2[:, :T * k].rearrange(
                            "p (j k) -> p j k", k=k).unsqueeze(1)
                            .broadcast_to([128, M, T, k]),
                        op=mybir.AluOpType.mult)
                # softmax over k (scores bounded; no max shift)
                nc.scalar.activation(sq_flat, sq_flat, AF.Exp)
                nc.vector.tensor_reduce(
                    out=den, in_=sq_,
                    op=mybir.AluOpType.add, axis=mybir.AxisListType.X)
                nc.vector.reciprocal_approx_fast(riv, den)
                nc.vector.tensor_tensor(
                    out=sq_, in0=sq_,
                    in1=riv.unsqueeze(2).broadcast_to([128, B, k]),
                    op=mybir.AluOpType.mult)
                # aggregate: PR = z * p ; tree over m ; + x_norm
                nc.vector.tensor_tensor(
                    out=PRb[:, :, :f].rearrange("p b (d k) -> p b d k", k=k),
                    in0=zq_dk,
                    in1=sq_.unsqueeze(2).broadcast_to([128, B, KD, k]),
                    op=mybir.AluOpType.mult)
                cm = M
                while cm > 1:
                    h = cm // 2
                    nc.vector.tensor_tensor(
                        out=PRm[:, :h, :, :f], in0=PRm[:, :h, :, :f],
                        in1=PRm[:, h:cm, :, :f], op=mybir.AluOpType.add)
                    cm = h
                nc.vector.tensor_tensor(
                    out=usl, in0=PRm[:, 0, :, :f], in1=xsl,
                    op=mybir.AluOpType.add)
                if t < cfg.routit - 1:
                    norm_stats(U2, c, k)
                else:
                    nc.scalar.activation(XC[:, t0:t0 + T, :f], usl, AF.Relu)

            NCH = len(cfg.ch_tiles)

            # ---- prologue: PCA + normalize + table + gathers for layer 0 ----
            for c in range(NCH):
                pca_chunk(c)
                normalize_chunk(XC, XN, c, cfg.caps[0])
                tshard_chunk(c)
                allgathers(c)
                if cfg.tbase[c] + cfg.ch_tiles[c] == AG_SPLIT:
                    gathers_early()
            gathers_rest()
            nc.sync.dma_start(
                outs_d[0].ap().rearrange("(j p) f -> p j f", p=128), XC)

            # ---- routing layers -------------------------------------------
            for li, k in enumerate(cfg.caps):
                f = k * KD
                for c in range(NCH):
                    for t in range(cfg.routit):
                        routing_iter(c, t, k)
                    if li < len(cfg.caps) - 1:
                        linear_chunk(li + 1, c)
                        normalize_chunk(U2, XN, c, cfg.caps[li + 1])
                        tshard_chunk(c)
                        allgathers(c)
                        if cfg.tbase[c] + cfg.ch_tiles[c] == AG_SPLIT:
                            gathers_early()
                if li < len(cfg.caps) - 1:
                    gathers_rest()
                nc.sync.dma_start(
                    outs_d[li + 1].ap().rearrange("(j p) f -> p j f", p=128),
                    XC[:, :, :f])

    nc.compile()
    return nc


# ----------------------------------------------------------------------------
# Host-side prep / assembly
# ----------------------------------------------------------------------------

def prepare_in_maps(cfg: Cfg, feature, neighbor_id, pca_w, pca_b, ws, bs_):
    NS, NP = cfg.nshard, cfg.np_
    nb = np.asarray(neighbor_id).astype(np.int64)
    perms = [perm_dk(k) for k in cfg.caps]
    p0 = perms[0]

    pwt = np.zeros((cfg.fpad, 128), np.float16)
    pwt[:cfg.feat, :] = np.asarray(pca_w).T[:, p0].astype(np.float16)
    pbb = np.zeros((128, 1), np.float32)
    pbb[:, 0] = np.asarray(pca_b, np.float32)[p0]
    wts, bss = [], []
    for i, (w, b) in enumerate(zip(ws, bs_)):
        fo, fi = w.shape
        wp = np.asarray(w)[perms[i + 1]][:, perms[i]]   # out-perm, in-perm
        wt = np.zeros((128, fo), np.float16)
        wt[:fi, :] = wp.T.astype(np.float16)
        wts.append(wt)
        bb = np.zeros((128, 1), np.float32)
        bb[:fo, 0] = np.asarray(b, np.float32)[perms[i + 1]]
        bss.append(bb)

    rows_a = cfg.rows_a
    base_b = cfg.base_b

    def table_row(G):
        c, g = np.divmod(G, NS)
        j, p = np.divmod(g, 128)
        ra = c * rows_a + j * 128 + p
        rb = base_b + c * cfg.rows_b + (j - AG_SPLIT) * 128 + p
        return np.where(j < AG_SPLIT, ra, rb)

    in_maps = []
    ng_per_chunk = [M * t * 128 // GCH for t in cfg.ch_tiles]
    nA = [[g for g in ng_per_chunk]]                  # upper bound
    for c in range(cfg.n_cores):
        lo = c * NS
        ft = np.zeros((cfg.fpad, NP), np.float16)
        ft[:cfg.feat, :NS] = np.asarray(feature[lo:lo + NS]).T.astype(np.float16)

        rows = np.zeros((NP, M), np.int64)
        rows[:NS] = table_row(nb[lo:lo + NS, :])
        # piece-A rows first within each node's m-slots (routing is
        # m-permutation invariant); lets leading gather groups be all-A
        order = np.argsort(rows >= base_b, axis=1, kind="stable")
        rows = np.take_along_axis(rows, order, axis=1)
        # gather order: chunk-major, then m, then tile (jj), then p
        parts = []
        na_core = []
        for ci in range(len(cfg.ch_tiles)):
            t0, T = int(cfg.tbase[ci]), cfg.ch_tiles[ci]
            r = rows[t0 * 128:(t0 + T) * 128, :].reshape(T * 128, M)
            flat = r.T.reshape(-1)                    # [m, jj*128]
            parts.append(flat)
            gmax = flat.reshape(-1, GCH).max(axis=1)
            allA = np.concatenate([gmax < base_b, [False]])
            na_core.append(int(np.argmin(allA)))
        nA.append(na_core)
        gidx = np.concatenate(parts).astype(np.int16)
        gidx_w = np.tile(gidx.reshape(-1, 16).T, (8, 1))

        m = {"feat_t": ft, "pca_wT": pwt, "pca_b": pbb, "gidx": gidx_w}
        for i in range(len(wts)):
            m[f"w{i + 1}T"] = wts[i]
            m[f"b{i + 1}"] = bss[i]
        in_maps.append(m)
    nA_final = [min(col) for col in zip(*nA)]
    return in_maps, nA_final


def assemble_output(cfg: Cfg, results):
    NS = cfg.nshard
    fdims = [128] + [k * KD for k in cfg.caps]
    perms = [perm_dk(k) for k in [8] + list(cfg.caps)]
    cols = []
    for li, f in enumerate(fdims):
        perm = perms[li]
        shards = []
        for c in range(cfg.n_cores):
            a = np.asarray(results[c][f"y{li}"]).astype(np.float32)[:NS]
            u = np.empty_like(a)
            u[:, perm] = a                                # undo (d,k) packing
            shards.append(u)
        cols.append(np.concatenate(shards, axis=0))
    return np.concatenate(cols, axis=1)


def _ensure_ntff_hook():
    try:
        from antenv.axon_hooks import get_axon_ntff_profile_hook  # noqa: F401
        return True
    except ImportError:
        pass
    try:
        import types
        import antenv
        from trn_agent_boot.trn_boot import _ntff_profile_via_ctypes
        mod = types.ModuleType("antenv.axon_hooks")
        state = {"h": None}
        mod.set_axon_ntff_profile_hook = lambda h: state.__setitem__("h", h)
        mod.get_axon_ntff_profile_hook = lambda: state["h"]
        sys.modules["antenv.axon_hooks"] = mod
        antenv.axon_hooks = mod
        mod.set_axon_ntff_profile_hook(
            _ntff_profile_via_ctypes("/opt/axon/libaxon_pjrt.so"))
        return True
    except Exception:
        return False


_CACHE = {}


def _build_with_unified_act_tables(cfg: Cfg):
    """Compile with Exp/Ln visible only in natural_log_exp_and_others.

    The act-table-load pass greedily picks the first set containing each
    function, which splits Exp (exp_and_others) from Ln (natural_log) and
    reloads tables every routing iteration.  Hiding Exp/Ln from the other
    sets (order and ids unchanged -- the id indexes the canonical list)
    makes it pick the one set that has both, so the inner loop runs with
    zero table reloads.  Restored right after compile.
    """
    orig = bacc.get_activation_tables
    target = "natural_log_exp_and_others"

    def patched(arch):
        tabs = orig(arch)
        if target in tabs:
            for name, fns in tabs.items():
                if name != target:
                    fns.discard(AF.Exp)
                    fns.discard(AF.Ln)
        return tabs

    bacc.get_activation_tables = patched
    try:
        return build_nc(cfg)
    finally:
        bacc.get_activation_tables = orig


def _get_nc(cfg: Cfg):
    key = (cfg.nshard, cfg.feat, cfg.n_cores, tuple(cfg.nA))
    if key not in _CACHE:
        _CACHE[key] = _build_with_unified_act_tables(cfg)
    return _CACHE[key]


def kernel(feature, neighbor_id, pca_w, pca_b,
           w1, b1, w2, b2, w3, b3, w4, b4, w5, b5):
    cfg = FULL_CFG
    in_maps, nA = prepare_in_maps(
        cfg, np.asarray(feature), np.asarray(neighbor_id),
        np.asarray(pca_w), np.asarray(pca_b),
        [np.asarray(w) for w in (w1, w2, w3, w4, w5)],
        [np.asarray(b) for b in (b1, b2, b3, b4, b5)])
    cfg.nA = nA
    nc = _get_nc(cfg)
    trace = bool(int(os.environ.get("KERNEL_TRACE", "0")))
    if trace:
        trace = _ensure_ntff_hook()
    tmpdir = os.environ.get("KERNEL_TRACE_DIR") or None
    res = run_bass_kernel_spmd(nc, in_maps, core_ids=list(range(cfg.n_cores)),
                               trace=trace, tmpdir=tmpdir)
    out = assemble_output(cfg, res.results)
    if trace:
        kernel.last_exec_time_ns = res.exec_time_ns
    return out


kernel.last_exec_time_ns = None


# revision 6
# speedup vs baseline: 1.2425x; 1.0446x over previous
"""DisenGCN Trainium2 kernel (8 NeuronCores, SPMD node-parallel).

Strategy (hardcoded from the problem spec):
  - Shard the 20000 nodes across 8 cores (2500/core, padded to 2560 = 20*128).
  - Weights replicated; per layer each core computes its local normalized
    embedding shard, AllGathers the full table to DRAM, then dma_gathers its
    neighbor rows into SBUF and runs the 5 capsule-routing iterations on the
    Vector/Scalar engines (node-major layout: nodes on partitions).
  - Features are stored in (d, k) transposed capsule order (host-side weight
    permutation) so per-capsule reductions are flat prefix-halving tree adds
    and all broadcast multiplies have contiguous innermost APs (DVE 2x mode).
  - Nodes are processed in 6 chunks of [2,4,4,4,4,2] tiles so the dma_gather
    descriptor generation (GPSIMD-bound, ~8us/1024 rows) of chunk c+1
    overlaps the routing DVE work of chunk c, and the per-layer tail
    (linear -> normalize -> tshard -> AllGather -> first gather) is short.
  - The AllGather is split in two pieces (tiles 0-17 / 18-19, j-major table
    rows) so the first piece overlaps the last chunks' routing.
  - 1/sqrt(ss) is computed as exp(-0.5*ln(ss+1e-12)) on the ACT engine: Ln,
    Exp, Square, Relu, Identity all live in one activation table set
    (natural_log_exp_and_others), so the inner loop never reloads tables.
  - fp16 storage/compute; fp32 only for softmax denominators and ln output.
"""

import os
import sys
import numpy as np

for _p in ("/opt/trn_rl_repo", "/root/.axon_site/_ro/trn_rl_repo"):
    if os.path.isdir(_p) and _p not in sys.path:
        sys.path.insert(0, _p)

import concourse.bass as bass  # noqa: E402
import concourse.tile as tile  # noqa: E402
from concourse import bacc, mybir  # noqa: E402
from concourse.bass_utils import run_bass_kernel_spmd  # noqa: E402
from concourse.library_config import mlp as mlp_lib  # noqa: E402
from concourse.masks import make_identity  # noqa: E402

FP16 = mybir.dt.float16
FP32 = mybir.dt.float32
I16 = mybir.dt.int16
AF = mybir.ActivationFunctionType

N_CORES = 8
M = 16          # neighbor fanout
KD = 16         # per-capsule dim
CAPS = [8, 7, 6, 5, 4, 3]
ROUTIT = 5
GCH = 1024      # dma_gather rows per instruction (>1024 overflows the ring)

CH_TILES = [1, 3, 4, 4, 4, 4]        # chunk sizes (tiles of 128 nodes)
AG_SPLITS = (12, 16)                 # table pieces: tiles [0,12), [12,16), [16,20)


def perm_dk(k):
    """new position d*k+kk  <-  old feature index kk*KD+d."""
    p = np.empty(k * KD, np.int64)
    for d in range(KD):
        for kk in range(k):
            p[d * k + kk] = kk * KD + d
    return p


class Cfg:
    def __init__(self, nshard, feat, n_cores=N_CORES, caps=CAPS, routit=ROUTIT):
        self.n_cores = n_cores
        self.nshard = nshard
        self.np_ = ((nshard + 127) // 128) * 128
        self.nt = self.np_ // 128
        self.E = self.np_ * M
        self.feat = feat
        self.fpad = ((feat + 127) // 128) * 128
        self.fchunks = self.fpad // 128
        self.caps = caps
        self.routit = routit
        self.kmax = max(caps)
        self.ntab = n_cores * self.np_
        assert sum(CH_TILES) == self.nt
        self.ch_tiles = CH_TILES
        self.tbase = np.concatenate([[0], np.cumsum(CH_TILES)])[:-1]
        self.boff = np.concatenate([[0], np.cumsum([M * t for t in CH_TILES])])
        self.tmax = max(CH_TILES)
        # j-major table rows in three AllGather pieces split at AG_SPLITS
        s1, s2 = AG_SPLITS
        self.rows_a1 = s1 * 128
        self.rows_a2 = (s2 - s1) * 128
        self.rows_b = (self.nt - s2) * 128
        self.base_a2 = n_cores * self.rows_a1        # first piece-A2 table row
        self.base_b = self.base_a2 + n_cores * self.rows_a2
        # nA1/nA2[c]: leading 1024-idx gather groups of chunk c whose indices
        # all fall in piece A1 / A1+A2 (host-verified); those can start
        # gathering right after the corresponding AllGather piece lands.
        self.nA1 = [0] * len(CH_TILES)
        self.nA2 = [0] * len(CH_TILES)


FULL_CFG = Cfg(2500, 500)


def build_nc(cfg: Cfg):
    nc = bacc.Bacc("TRN2", target_bir_lowering=False, debug=False,
                   num_devices=cfg.n_cores)
    NT, NP, E = cfg.nt, cfg.np_, cfg.E
    KM = cfg.kmax
    TM = cfg.tmax
    BM = M * TM

    feat_t = nc.dram_tensor("feat_t", [cfg.fpad, NP], FP16, kind="ExternalInput")
    pca_wT = nc.dram_tensor("pca_wT", [cfg.fpad, 128], FP16, kind="ExternalInput")
    pca_b = nc.dram_tensor("pca_b", [128, 1], FP32, kind="ExternalInput")
    wTs, bs = [], []
    for i in range(1, len(cfg.caps)):
        fo = cfg.caps[i] * KD
        wTs.append(nc.dram_tensor(f"w{i}T", [128, fo], FP16, kind="ExternalInput"))
        bs.append(nc.dram_tensor(f"b{i}", [128, 1], FP32, kind="ExternalInput"))
    gidx_d = nc.dram_tensor("gidx", [128, E // 16], I16, kind="ExternalInput")
    outs_d = []
    fdims = [128] + [k * KD for k in cfg.caps]
    for li, f in enumerate(fdims):
        outs_d.append(nc.dram_tensor(f"y{li}", [NP, f], FP16, kind="ExternalOutput"))

    tshard = nc.dram_tensor("tshard", [NP, 128], FP16)
    table = nc.dram_tensor("table", [cfg.ntab, 128], FP16, addr_space="Shared")

    def sb(name, shape, dt):
        return nc.alloc_sbuf_tensor(name, shape, dt).ap()

    Z = sb("Z", [128, M * NT, 128], FP16)       # chunk-major [c][m][jj] blocks
    XC = sb("XC", [128, NT, 128], FP16)
    XN = sb("XN", [128, NT, 128], FP16)
    U2 = sb("U2", [128, NT, 128], FP16)
    XT = sb("XT", [128, NP], FP16)
    XLT = sb("XLT", [128, NP], FP16)
    PR = sb("PR", [128, BM, 128], FP16)         # per-chunk scratch
    S = sb("S", [128, BM * KM], FP16)           # scores -> exp -> p (in place)
    DEN = sb("DEN", [128, BM], FP32)
    RIV = sb("RIV", [128, BM], FP32)
    SQ = sb("SQ", [128, TM, 128], FP16)         # normalize scratch
    RIN = sb("RIN", [128, TM * KM], FP16)
    LNS = sb("LNS", [128, TM * KM], FP32)
    RIN2 = sb("RIN2", [128, TM * KM], FP16)
    EPS = sb("EPS", [128, 1], FP32)
    GIDX = sb("GIDX", [128, E // 16], I16)
    IDT = sb("IDT", [128, 128], FP16)
    FT = sb("FT", [128, cfg.fchunks, NP], FP16)
    PW = sb("PW", [128, cfg.fchunks, 128], FP16)
    PB = sb("PB", [128, 1], FP32)
    WTS = [sb(f"WTS{i}", [128, cfg.caps[i] * KD], FP16)
           for i in range(1, len(cfg.caps))]
    BS = [sb(f"BS{i}", [128, 1], FP32) for i in range(1, len(cfg.caps))]

    Zb = Z.rearrange("p (b f) -> p b f", f=128) if Z.ndim == 2 else Z

    with tile.TileContext(nc) as tc:
        import contextlib
        ctx = contextlib.ExitStack()
        with ctx:
            psum = ctx.enter_context(tc.tile_pool(name="psum", bufs=2, space="PSUM"))
            nc.gpsimd.load_library(mlp_lib)
            make_identity(nc, IDT)
            nc.vector.memset(EPS, 1e-12)

            nc.sync.dma_start(GIDX, gidx_d.ap())
            nc.sync.dma_start(FT, feat_t.ap().rearrange("(c p) n -> p c n", p=128))
            nc.sync.dma_start(PW, pca_wT.ap().rearrange("(c p) f -> p c f", p=128))
            nc.sync.dma_start(PB, pca_b.ap())
            for i in range(len(cfg.caps) - 1):
                nc.sync.dma_start(WTS[i], wTs[i].ap())
                nc.sync.dma_start(BS[i], bs[i].ap())

            def transpose_block(dst_ap, src_ap, fin, fout):
                pt = psum.tile([128, 128], FP16, tag="pt")
                nc.tensor.transpose(pt[:fout, :fin], src_ap, IDT[:fin, :fin])
                nc.scalar.copy(dst_ap, pt[:fout, :fin])

            def pca_chunk(c):
                t0, T = cfg.tbase[c], cfg.ch_tiles[c]
                cs, ce = t0 * 128, (t0 + T) * 128
                pl = psum.tile([128, TM * 128], FP32, tag="pl")
                for q in range(cfg.fchunks):
                    nc.tensor.matmul(
                        pl[:, :T * 128], PW[:, q, :], FT[:, q, cs:ce],
                        start=(q == 0), stop=(q == cfg.fchunks - 1))
                nc.scalar.activation(XLT[:, cs:ce], pl[:, :T * 128],
                                     AF.Relu, bias=PB[:, :], scale=1.0)
                for j in range(t0, t0 + T):
                    transpose_block(XC[:, j, :], XLT[:, j * 128:(j + 1) * 128],
                                    128, 128)

            def linear_chunk(li, c):
                # target layer li (1..5): XC (fin) -> U2 (fout)
                fin = cfg.caps[li - 1] * KD
                fout = cfg.caps[li] * KD
                t0, T = cfg.tbase[c], cfg.ch_tiles[c]
                cs, ce = t0 * 128, (t0 + T) * 128
                for j in range(t0, t0 + T):
                    transpose_block(XT[:fin, j * 128:(j + 1) * 128],
                                    XC[:, j, :fin], 128, fin)
                pl = psum.tile([128, TM * 128], FP32, tag="pl")
                nc.tensor.matmul(pl[:fout, :T * 128], WTS[li - 1][:fin, :fout],
                                 XT[:fin, cs:ce], start=True, stop=True)
                nc.scalar.activation(XLT[:fout, cs:ce], pl[:fout, :T * 128],
                                     AF.Identity, bias=BS[li - 1][:fout, :],
                                     scale=1.0)
                for j in range(t0, t0 + T):
                    transpose_block(U2[:, j, :fout], XLT[:fout, j * 128:(j + 1) * 128],
                                    fout, 128)

            def norm_stats(src, c, k):
                """RIN2 = 1/||src_chunk|| per capsule: exp(-0.5*ln(ss+eps))."""
                f = k * KD
                t0, T = cfg.tbase[c], cfg.ch_tiles[c]
                nc.scalar.activation(SQ[:, :T, :f], src[:, t0:t0 + T, :f],
                                     AF.Square)
                cur = f
                rin = RIN[:, :T * k]
                while cur > k:
                    h = cur // 2
                    if h > k:
                        nc.vector.tensor_tensor(
                            out=SQ[:, :T, :h], in0=SQ[:, :T, :h],
                            in1=SQ[:, :T, h:cur], op=mybir.AluOpType.add)
                    else:
                        nc.vector.tensor_tensor(
                            out=rin.rearrange("p (j k) -> p j k", k=k),
                            in0=SQ[:, :T, :h], in1=SQ[:, :T, h:cur],
                            op=mybir.AluOpType.add)
                    cur = h
                nc.scalar.activation(LNS[:, :T * k], rin, AF.Ln,
                                     bias=EPS[:, :], scale=1.0)
                nc.scalar.activation(RIN2[:, :T * k], LNS[:, :T * k],
                                     AF.Exp, scale=-0.5)

            def normalize_chunk(src, dst, c, k):
                """dst = per-capsule l2 normalize of src for chunk c."""
                f = k * KD
                t0, T = cfg.tbase[c], cfg.ch_tiles[c]
                norm_stats(src, c, k)
                nc.vector.tensor_tensor(
                    out=dst[:, t0:t0 + T, :f].rearrange(
                        "p j (d k) -> p j d k", k=k),
                    in0=src[:, t0:t0 + T, :f].rearrange(
                        "p j (d k) -> p j d k", k=k),
                    in1=RIN2[:, :T * k].rearrange("p (j k) -> p j k", k=k)
                        .unsqueeze(2).broadcast_to([128, T, KD, k]),
                    op=mybir.AluOpType.mult)

            def tshard_chunk(c):
                t0, T = cfg.tbase[c], cfg.ch_tiles[c]
                nc.sync.dma_start(
                    tshard.ap()[t0 * 128:(t0 + T) * 128, :]
                    .rearrange("(j p) f -> p j f", p=128),
                    XN[:, t0:t0 + T, :])

            def ag_piece(r0, r1, t0, t1):
                nc.gpsimd.collective_compute(
                    "AllGather", mybir.AluOpType.bypass,
                    replica_groups=[list(range(cfg.n_cores))],
                    ins=[tshard.ap()[r0:r1, :]],
                    outs=[table.ap()[t0:t1, :]])

            def allgathers(c):
                cum = cfg.tbase[c] + cfg.ch_tiles[c]
                if cum == AG_SPLITS[0]:
                    ag_piece(0, cfg.rows_a1, 0, cfg.base_a2)
                elif cum == AG_SPLITS[1]:
                    ag_piece(cfg.rows_a1, cfg.rows_a1 + cfg.rows_a2,
                             cfg.base_a2, cfg.base_b)
                elif cum == NT:
                    ag_piece(cfg.rows_a1 + cfg.rows_a2, NP,
                             cfg.base_b, cfg.ntab)

            def gather_group(c, g, rmax):
                b0 = int(cfg.boff[c])
                gb = GCH // 128
                i0 = (b0 + g * gb) * 128
                src_ap = table.ap() if rmax is None else table.ap()[0:rmax, :]
                nc.gpsimd.dma_gather(
                    Zb[:, b0 + g * gb:b0 + (g + 1) * gb, :],
                    src_ap,
                    GIDX[:, i0 // 16:(i0 + GCH) // 16],
                    GCH, GCH, 128)

            # Early gather phases: prefix groups whose rows all sit in the
            # already-AllGathered table pieces (host-verified); they may only
            # write Z chunks whose routing is already program-before, i.e.
            # chunks up to the one whose tail triggered the AG piece.
            def gathers_early1():
                for c in range(4):                   # chunks 0..3 (tiles 0..11)
                    for g in range(cfg.nA1[c]):
                        gather_group(c, g, cfg.base_a2)

            def gathers_early2():
                for c in range(5):                   # chunks 0..4 (tiles 0..15)
                    g0 = cfg.nA1[c] if c < 4 else 0
                    for g in range(g0, cfg.nA2[c]):
                        gather_group(c, g, cfg.base_b)

            def gathers_rest():
                for c in range(len(cfg.ch_tiles)):
                    ng = M * cfg.ch_tiles[c] * 128 // GCH
                    g0 = cfg.nA2[c] if c < 5 else 0
                    for g in range(g0, ng):
                        gather_group(c, g, None)

            def routing_iter(c, t, k):
                f = k * KD
                t0, T = cfg.tbase[c], cfg.ch_tiles[c]
                B = M * T
                b0 = int(cfg.boff[c])
                zq = Zb[:, b0:b0 + B, :f]
                zq_m = zq.rearrange("p (m j) f -> p m j f", m=M)
                zq_dk = zq.rearrange("p b (d k) -> p b d k", k=k)
                sq_ = S[:, :B * k].rearrange("p (b k) -> p b k", k=k)
                sq_flat = S[:, :B * k]
                sq4 = S[:, :B * k].rearrange("p (m j k) -> p m j k", m=M, k=k)
                PRb = PR[:, :B, :]
                PRm = PR[:, :B, :].rearrange("p (m j) f -> p m j f", m=M)
                den = DEN[:, :B]
                riv = RIV[:, :B]
                xsl = XN[:, t0:t0 + T, :f]
                usl = U2[:, t0:t0 + T, :f]
                usrc = xsl if t == 0 else usl
                # scores: PR = z * u ; tree-reduce over d -> S
                nc.vector.tensor_tensor(
                    out=PRm[:, :, :, :f], in0=zq_m,
                    in1=usrc.unsqueeze(1).broadcast_to([128, M, T, f]),
                    op=mybir.AluOpType.mult)
                cur = f
                while cur > k:
                    h = cur // 2
                    dst = PRb[:, :, :h] if h > k else sq_
                    nc.vector.tensor_tensor(
                        out=dst, in0=PRb[:, :, :h],
                        in1=PRb[:, :, h:cur], op=mybir.AluOpType.add)
                    cur = h
                if t > 0:
                    # u was left unnormalized; scale scores by 1/||u||
                    nc.vector.tensor_tensor(
                        out=sq4, in0=sq4,
                        in1=RIN2[:, :T * k].rearrange(
                            "p (j k) -> p j k", k=k).unsqueeze(1)
                            .broadcast_to([128, M, T, k]),
                        op=mybir.AluOpType.mult)
                # softmax over k (scores bounded; no max shift)
                nc.scalar.activation(sq_flat, sq_flat, AF.Exp)
                nc.vector.tensor_reduce(
                    out=den, in_=sq_,
                    op=mybir.AluOpType.add, axis=mybir.AxisListType.X)
                nc.vector.reciprocal_approx_fast(riv, den)
                nc.vector.tensor_tensor(
                    out=sq_, in0=sq_,
                    in1=riv.unsqueeze(2).broadcast_to([128, B, k]),
                    op=mybir.AluOpType.mult)
                # aggregate: PR = z * p ; tree over m ; + x_norm
                nc.vector.tensor_tensor(
                    out=PRb[:, :, :f].rearrange("p b (d k) -> p b d k", k=k),
                    in0=zq_dk,
                    in1=sq_.unsqueeze(2).broadcast_to([128, B, KD, k]),
                    op=mybir.AluOpType.mult)
                cm = M
                while cm > 1:
                    h = cm // 2
                    nc.vector.tensor_tensor(
                        out=PRm[:, :h, :, :f], in0=PRm[:, :h, :, :f],
                        in1=PRm[:, h:cm, :, :f], op=mybir.AluOpType.add)
                    cm = h
                nc.vector.tensor_tensor(
                    out=usl, in0=PRm[:, 0, :, :f], in1=xsl,
                    op=mybir.AluOpType.add)
                if t < cfg.routit - 1:
                    norm_stats(U2, c, k)
                else:
                    nc.scalar.activation(XC[:, t0:t0 + T, :f], usl, AF.Relu)

            NCH = len(cfg.ch_tiles)

            # ---- prologue: PCA + normalize + table + gathers for layer 0 ----
            for c in range(NCH):
                pca_chunk(c)
                normalize_chunk(XC, XN, c, cfg.caps[0])
                tshard_chunk(c)
                allgathers(c)
                if cfg.tbase[c] + cfg.ch_tiles[c] == AG_SPLITS[0]:
                    gathers_early1()
                elif cfg.tbase[c] + cfg.ch_tiles[c] == AG_SPLITS[1]:
                    gathers_early2()
            gathers_rest()
            nc.sync.dma_start(
                outs_d[0].ap().rearrange("(j p) f -> p j f", p=128), XC)

            # ---- routing layers -------------------------------------------
            for li, k in enumerate(cfg.caps):
                f = k * KD
                for c in range(NCH):
                    for t in range(cfg.routit):
                        routing_iter(c, t, k)
                    if li < len(cfg.caps) - 1:
                        linear_chunk(li + 1, c)
                        normalize_chunk(U2, XN, c, cfg.caps[li + 1])
                        tshard_chunk(c)
                        allgathers(c)
                        if cfg.tbase[c] + cfg.ch_tiles[c] == AG_SPLITS[0]:
                            gathers_early1()
                        elif cfg.tbase[c] + cfg.ch_tiles[c] == AG_SPLITS[1]:
                            gathers_early2()
                if li < len(cfg.caps) - 1:
                    gathers_rest()
                nc.sync.dma_start(
                    outs_d[li + 1].ap().rearrange("(j p) f -> p j f", p=128),
                    XC[:, :, :f])

    nc.compile()
    return nc


# ----------------------------------------------------------------------------
# Host-side prep / assembly
# ----------------------------------------------------------------------------

def prepare_in_maps(cfg: Cfg, feature, neighbor_id, pca_w, pca_b, ws, bs_):
    NS, NP = cfg.nshard, cfg.np_
    nb = np.asarray(neighbor_id).astype(np.int64)
    perms = [perm_dk(k) for k in cfg.caps]
    p0 = perms[0]

    pwt = np.zeros((cfg.fpad, 128), np.float16)
    pwt[:cfg.feat, :] = np.asarray(pca_w).T[:, p0].astype(np.float16)
    pbb = np.zeros((128, 1), np.float32)
    pbb[:, 0] = np.asarray(pca_b, np.float32)[p0]
    wts, bss = [], []
    for i, (w, b) in enumerate(zip(ws, bs_)):
        fo, fi = w.shape
        wp = np.asarray(w)[perms[i + 1]][:, perms[i]]   # out-perm, in-perm
        wt = np.zeros((128, fo), np.float16)
        wt[:fi, :] = wp.T.astype(np.float16)
        wts.append(wt)
        bb = np.zeros((128, 1), np.float32)
        bb[:fo, 0] = np.asarray(b, np.float32)[perms[i + 1]]
        bss.append(bb)

    s1, s2 = AG_SPLITS
    base_a2, base_b = cfg.base_a2, cfg.base_b

    def table_row(G):
        c, g = np.divmod(G, NS)
        j, p = np.divmod(g, 128)
        r1 = c * cfg.rows_a1 + j * 128 + p
        r2 = base_a2 + c * cfg.rows_a2 + (j - s1) * 128 + p
        r3 = base_b + c * cfg.rows_b + (j - s2) * 128 + p
        return np.where(j < s1, r1, np.where(j < s2, r2, r3))

    in_maps = []
    ng_per_chunk = [M * t * 128 // GCH for t in cfg.ch_tiles]
    nA1 = [[g for g in ng_per_chunk]]                 # upper bound
    nA2 = [[g for g in ng_per_chunk]]
    for c in range(cfg.n_cores):
        lo = c * NS
        ft = np.zeros((cfg.fpad, NP), np.float16)
        ft[:cfg.feat, :NS] = np.asarray(feature[lo:lo + NS]).T.astype(np.float16)

        rows = np.zeros((NP, M), np.int64)
        rows[:NS] = table_row(nb[lo:lo + NS, :])
        # earlier-piece rows first within each node's m-slots (routing is
        # m-permutation invariant); lets leading gather groups be all-A1/A2
        piece = (rows >= base_a2).astype(np.int8) + (rows >= base_b)
        order = np.argsort(piece, axis=1, kind="stable")
        rows = np.take_along_axis(rows, order, axis=1)
        # gather order: chunk-major, then m, then tile (jj), then p
        parts = []
        na1_core, na2_core = [], []
        for ci in range(len(cfg.ch_tiles)):
            t0, T = int(cfg.tbase[ci]), cfg.ch_tiles[ci]
            r = rows[t0 * 128:(t0 + T) * 128, :].reshape(T * 128, M)
            flat = r.T.reshape(-1)                    # [m, jj*128]
            parts.append(flat)
            gmax = flat.reshape(-1, GCH).max(axis=1)
            a1 = np.concatenate([gmax < base_a2, [False]])
            a2 = np.concatenate([gmax < base_b, [False]])
            na1_core.append(int(np.argmin(a1)))
            na2_core.append(int(np.argmin(a2)))
        nA1.append(na1_core)
        nA2.append(na2_core)
        gidx = np.concatenate(parts).astype(np.int16)
        gidx_w = np.tile(gidx.reshape(-1, 16).T, (8, 1))

        m = {"feat_t": ft, "pca_wT": pwt, "pca_b": pbb, "gidx": gidx_w}
        for i in range(len(wts)):
            m[f"w{i + 1}T"] = wts[i]
            m[f"b{i + 1}"] = bss[i]
        in_maps.append(m)
    nA1_final = [min(col) for col in zip(*nA1)]
    nA2_final = [min(col) for col in zip(*nA2)]
    return in_maps, nA1_final, nA2_final


def assemble_output(cfg: Cfg, results):
    NS = cfg.nshard
    fdims = [128] + [k * KD for k in cfg.caps]
    perms = [perm_dk(k) for k in [8] + list(cfg.caps)]
    cols = []
    for li, f in enumerate(fdims):
        perm = perms[li]
        shards = []
        for c in range(cfg.n_cores):
            a = np.asarray(results[c][f"y{li}"]).astype(np.float32)[:NS]
            u = np.empty_like(a)
            u[:, perm] = a                                # undo (d,k) packing
            shards.append(u)
        cols.append(np.concatenate(shards, axis=0))
    return np.concatenate(cols, axis=1)


def _ensure_ntff_hook():
    try:
        from antenv.axon_hooks import get_axon_ntff_profile_hook  # noqa: F401
        return True
    except ImportError:
        pass
    try:
        import types
        import antenv
        from trn_agent_boot.trn_boot import _ntff_profile_via_ctypes
        mod = types.ModuleType("antenv.axon_hooks")
        state = {"h": None}
        mod.set_axon_ntff_profile_hook = lambda h: state.__setitem__("h", h)
        mod.get_axon_ntff_profile_hook = lambda: state["h"]
        sys.modules["antenv.axon_hooks"] = mod
        antenv.axon_hooks = mod
        mod.set_axon_ntff_profile_hook(
            _ntff_profile_via_ctypes("/opt/axon/libaxon_pjrt.so"))
        return True
    except Exception:
        return False


_CACHE = {}


def _build_with_unified_act_tables(cfg: Cfg):
    """Compile with Exp/Ln visible only in natural_log_exp_and_others.

    The act-table-load pass greedily picks the first set containing each
    function, which splits Exp (exp_and_others) from Ln (natural_log) and
    reloads tables every routing iteration.  Hiding Exp/Ln from the other
    sets (order and ids unchanged -- the id indexes the canonical list)
    makes it pick the one set that has both, so the inner loop runs with
    zero table reloads.  Restored right after compile.
    """
    orig = bacc.get_activation_tables
    target = "natural_log_exp_and_others"

    def patched(arch):
        tabs = orig(arch)
        if target in tabs:
            for name, fns in tabs.items():
                if name != target:
                    fns.discard(AF.Exp)
                    fns.discard(AF.Ln)
        return tabs

    bacc.get_activation_tables = patched
    try:
        return build_nc(cfg)
    finally:
        bacc.get_activation_tables = orig


def _get_nc(cfg: Cfg):
    key = (cfg.nshard, cfg.feat, cfg.n_cores, tuple(cfg.nA1), tuple(cfg.nA2))
    if key not in _CACHE:
        _CACHE[key] = _build_with_unified_act_tables(cfg)
    return _CACHE[key]


def kernel(feature, neighbor_id, pca_w, pca_b,
           w1, b1, w2, b2, w3, b3, w4, b4, w5, b5):
    cfg = FULL_CFG
    in_maps, nA1, nA2 = prepare_in_maps(
        cfg, np.asarray(feature), np.asarray(neighbor_id),
        np.asarray(pca_w), np.asarray(pca_b),
        [np.asarray(w) for w in (w1, w2, w3, w4, w5)],
        [np.asarray(b) for b in (b1, b2, b3, b4, b5)])
    cfg.nA1, cfg.nA2 = nA1, nA2
    nc = _get_nc(cfg)
    trace = bool(int(os.environ.get("KERNEL_TRACE", "0")))
    if trace:
        trace = _ensure_ntff_hook()
    tmpdir = os.environ.get("KERNEL_TRACE_DIR") or None
    res = run_bass_kernel_spmd(nc, in_maps, core_ids=list(range(cfg.n_cores)),
                               trace=trace, tmpdir=tmpdir)
    out = assemble_output(cfg, res.results)
    if trace:
        kernel.last_exec_time_ns = res.exec_time_ns
    return out


kernel.last_exec_time_ns = None
